# revision 49
# baseline (speedup 1.0000x reference)
"""Trainium2 Bass kernel for nn_AutoregressiveSender (GRU decoder + Gumbel-ST).

Self-contained: host-side prep (numpy) + SPMD Bass/Tile kernel on 8 NeuronCores.
Data parallel over batch (1024 rows/core), weights replicated.

Math notes vs reference:
- prev = token @ W_tok + b_tok is folded into the gate matmuls via
  P_g = W_tok @ Wg[:H] (6xH) and per-step bias vectors (host-precomputed),
  cutting gate contractions from 2H to H (+ a K=6 matmul).
- token == alpha * onehot(argmax(logits+g)) exactly, with
  alpha = (1-s)+s, s = 1/sum(exp(a-max)). Computed on-device.
- MM_MODE selects matmul precision:
    "f32"    exact fp32 (4 cyc/row, slowest)
    "bf16x3" hi/lo bf16 split, 3 passes (~5e-6 rel err)
    "f32r"   single-pass fp32r (~1.5e-4 rel err, fastest)
"""
import sys

for _p in ("/opt/trn_rl_repo", "/opt/pypackages"):
    if _p not in sys.path:
        sys.path.append(_p)

import numpy as np
import ml_dtypes
from contextlib import ExitStack

import concourse.tile as tile
from concourse import bacc, mybir
from concourse import bass_utils
from concourse.bass import _add_dep_helper

F32 = mybir.dt.float32
F32R = mybir.dt.float32r
BF16 = mybir.dt.bfloat16
AF = mybir.ActivationFunctionType
ALU = mybir.AluOpType

B, D, H, V, T = 8192, 2048, 1024, 6, 6
NC = 8
BL = B // NC          # 1024 rows per core
KD = D // 128         # 16 k-chunks (encoder)
KH = H // 128         # 8 k-chunks / m-chunks
BH = 2                # batch halves of 512
NB = BL // 2          # 512
NCH = BL // 128       # 8 batch chunks of 128

MM_MODE = "f32r"

_BUILD_CACHE = {}


def _split_bf16(a):
    hi = a.astype(ml_dtypes.bfloat16)
    lo = (a - hi.astype(np.float32)).astype(ml_dtypes.bfloat16)
    return hi, lo


def _round_f32r(a):
    """fp32r storage rounding: RNE to 11 mantissa bits (HW-verified)."""
    b = a.astype(np.float32).view(np.uint32).astype(np.uint64)
    shift = 12
    lsb = (b >> shift) & 1
    r = (b + ((1 << (shift - 1)) - 1 + lsb)) & ~np.uint64((1 << shift) - 1)
    return r.astype(np.uint32).view(np.float32)


def build(mode=MM_MODE):
    if mode in _BUILD_CACHE:
        return _BUILD_CACHE[mode]
    nc = bacc.Bacc("TRN2", target_bir_lowering=False, debug=False, num_devices=NC)
    dt_w = {"f32": F32, "bf16x3": BF16, "f32r": F32R}[mode]

    def din(name, shape, dt=F32):
        return nc.dram_tensor(name, shape, dt, kind="ExternalInput").ap()

    # per-core inputs
    if mode == "bf16x3":
        xTh_d = din("xTh", [D, BL], BF16)
        xTl_d = din("xTl", [D, BL], BF16)
    else:
        xT_d = din("xT", [D, BL], F32R if mode == "f32r" else F32)
    g_d = din("g", [128, T * 48])
    # replicated weights
    if mode == "bf16x3":
        wenc_h_d = din("Wenc_h", [D, H], BF16)
        wenc_l_d = din("Wenc_l", [D, H], BF16)
        gw_d = {g: (din(f"W{g}_h", [H, H], BF16), din(f"W{g}_l", [H, H], BF16))
                for g in "zrh"}
        p_d = {g: (din(f"P{g}_h", [V, H], BF16), din(f"P{g}_l", [V, H], BF16))
               for g in "zrh"}
        wout_h_d = din("Wout_h", [128, KH * V], BF16)
        wout_l_d = din("Wout_l", [128, KH * V], BF16)
    else:
        dt_in = F32R if mode == "f32r" else F32
        wenc_d = din("Wenc", [D, H], dt_in)
        gw_d = {g: din(f"W{g}", [H, H], dt_in) for g in "zrh"}
        p_d = {g: din(f"P{g}", [V, H], BF16 if mode == "f32r" else F32)
               for g in "zrh"}
        wout_d = din("Wout", [128, KH * V], dt_in)
    bias_d = din("bias", [128, 48])
    benc_d = din("benc", [128, KD // 2])
    bout_d = din("bout", [V, 1])
    iota_d = din("iota", [128, 48])
    eye128_d = din("eye128", [128, 128])
    eye6_d = din("eye6", [V, V])

    msg_d = nc.dram_tensor("msg", [BL, T * V], F32, kind="ExternalOutput").ap()
    log_d = nc.dram_tensor("logits", [T, BL, V], F32, kind="ExternalOutput").ap()
    ntok_d = nc.dram_tensor("ntok", [BL], F32, kind="ExternalOutput").ap()

    cast_dma = nc.sync  # f32r inputs are pre-rounded on host; no casting DMA
    dt_p = BF16 if mode == "f32r" else dt_w  # prev-token path dtype

    with tile.TileContext(nc) as tc, ExitStack() as ctx:
        # ---------- persistent SBUF ----------
        cons = ctx.enter_context(tc.tile_pool(name="cons", bufs=1))

        g_sb = cons.tile([128, T * 48], F32, tag="g")
        nc.sync.dma_start(g_sb[:], g_d)
        bias_sb = cons.tile([128, 48], F32, tag="bias")
        nc.sync.dma_start(bias_sb[:], bias_d)
        benc_sb = cons.tile([128, KD // 2], F32, tag="benc")
        nc.sync.dma_start(benc_sb[:], benc_d)
        bout_sb = cons.tile([V, 1], F32, tag="bout")
        nc.sync.dma_start(bout_sb[:], bout_d)
        iota_sb = cons.tile([128, 48], F32, tag="iota")
        nc.sync.dma_start(iota_sb[:], iota_d)
        eye128_sb = cons.tile([128, 128], F32, tag="eye128")
        nc.sync.dma_start(eye128_sb[:], eye128_d)
        eye6_sb = cons.tile([V, V], F32, tag="eye6")
        nc.sync.dma_start(eye6_sb[:], eye6_d)

        # state tiles
        st = ctx.enter_context(tc.tile_pool(name="state", bufs=1))
        h_sb = st.tile([128, KH, BL], F32, tag="h")
        rc = st.tile([128, NCH], F32, tag="rc")
        nt = st.tile([128, NCH], F32, tag="nt")
        msg_sb = st.tile([128, NCH, T * V], F32, tag="msg")
        tokT = st.tile([V, BL], dt_p, tag="tokT")
        nc.vector.memset(rc[:], 1.0)
        nc.vector.memset(nt[:], 0.0)

        # ---------- encoder ----------
        # W_enc is loaded into SBUF once (row-contiguous DMAs) and reused for
        # both batch halves; xT chunks stream per half.
        KHALF = KD // 2
        with tc.tile_pool(name="encps", bufs=KH, space="PSUM") as encps, \
             tc.tile_pool(name="encx", bufs=12) as encx, \
             tc.tile_pool(name="encw", bufs=2) as encw:
            if mode == "bf16x3":
                wt_h = [encw.tile([128, KHALF, H], BF16, tag="weh", name=f"weh{h}")
                        for h in range(2)]
                wt_l = [encw.tile([128, KHALF, H], BF16, tag="wel", name=f"wel{h}")
                        for h in range(2)]

                def load_wrow(half, kk):
                    k = half * KHALF + kk
                    nc.sync.dma_start(wt_h[half][:, kk, :],
                                      wenc_h_d[k * 128:(k + 1) * 128, :])
                    nc.sync.dma_start(wt_l[half][:, kk, :],
                                      wenc_l_d[k * 128:(k + 1) * 128, :])
            else:
                wts_enc = [encw.tile([128, KHALF, H], dt_w, tag="we", name=f"we{h}")
                           for h in range(2)]

                def load_wrow(half, kk):
                    k = half * KHALF + kk
                    for hh2 in range(2):
                        cast_dma.dma_start(
                            wts_enc[half][:, kk, hh2 * (H // 2):(hh2 + 1) * (H // 2)],
                            wenc_d[k * 128:(k + 1) * 128,
                                   hh2 * (H // 2):(hh2 + 1) * (H // 2)])

            for kk in range(KHALF):
                load_wrow(0, kk)
            for half in range(2):
                k0 = half * KHALF
                for bh in range(BH):
                    pts = [encps.tile([128, NB], F32, tag="enc",
                                      name=f"encp{half}_{bh}_{i}") for i in range(KH)]
                    for kk in range(KHALF):
                        k = k0 + kk
                        if mode == "bf16x3":
                            xh = encx.tile([128, NB], BF16, tag="xh")
                            nc.sync.dma_start(
                                xh[:], xTh_d[k * 128:(k + 1) * 128, bh * NB:(bh + 1) * NB])
                            xl = encx.tile([128, NB], BF16, tag="xl")
                            last_enc_dma = nc.sync.dma_start(
                                xl[:], xTl_d[k * 128:(k + 1) * 128, bh * NB:(bh + 1) * NB])
                        else:
                            xk = encx.tile([128, NB], dt_w, tag="x")
                            last_enc_dma = cast_dma.dma_start(
                                xk[:], xT_d[k * 128:(k + 1) * 128, bh * NB:(bh + 1) * NB])
                        if half == 0 and bh == 1:
                            load_wrow(1, kk)   # prefetch half1 weights in-stream
                        for m in range(KH):
                            first = (kk == 0)
                            last = (kk == KHALF - 1)
                            ms = slice(m * 128, (m + 1) * 128)
                            if mode == "bf16x3":
                                nc.tensor.matmul(pts[m][:], wt_h[half][:, kk, ms], xh[:], start=first, stop=False)
                                nc.tensor.matmul(pts[m][:], wt_h[half][:, kk, ms], xl[:], start=False, stop=False)
                                nc.tensor.matmul(pts[m][:], wt_l[half][:, kk, ms], xh[:], start=False, stop=last)
                            else:
                                nc.tensor.matmul(pts[m][:], wts_enc[half][:, kk, ms], xk[:], start=first, stop=last)
                    for m in range(KH):
                        hdst = h_sb[:, m, bh * NB:(bh + 1) * NB]
                        if half == 0:
                            # split evacs across ACT/DVE to halve the
                            # PSUM-free latency between encoder sub-phases
                            if m % 2 == 0:
                                nc.scalar.activation(hdst, pts[m][:], AF.Identity,
                                                     bias=benc_sb[:, m:m + 1])
                            else:
                                nc.vector.scalar_tensor_tensor(
                                    hdst, pts[m][:], 1.0, benc_sb[:, m:m + 1]
                                    .broadcast_to((128, NB)),
                                    op0=ALU.mult, op1=ALU.add)
                        else:
                            nc.vector.tensor_tensor(hdst, hdst, pts[m][:], op=ALU.add)

        wpool = ctx.enter_context(tc.tile_pool(name="wts", bufs=1))
        # GRU weights resident: [p, k, m] with k = contraction chunk.
        # Loaded in phase order (r, h, z) and gated behind the encoder's DMAs
        # so they don't steal HBM bandwidth from the encoder's working set.
        def after_enc(inst):
            _add_dep_helper(inst.ins, last_enc_dma.ins, sync=True,
                            reason="gate weights after encoder DMA")
            return inst

        gw = {}
        for g in "rhz":
            if mode == "bf16x3":
                wh = wpool.tile([128, KH, H], BF16, tag=f"W{g}h")
                wl = wpool.tile([128, KH, H], BF16, tag=f"W{g}l")
                after_enc(nc.sync.dma_start(
                    wh[:], gw_d[g][0].rearrange("(k p) m -> p k m", p=128)))
                after_enc(nc.sync.dma_start(
                    wl[:], gw_d[g][1].rearrange("(k p) m -> p k m", p=128)))
                gw[g] = (wh, wl)
            else:
                w = wpool.tile([128, KH, H], dt_w, tag=f"W{g}")
                after_enc(cast_dma.dma_start(
                    w[:], gw_d[g].rearrange("(k p) m -> p k m", p=128)))
                gw[g] = w
        pw = {}
        for g in "zrh":
            if mode == "bf16x3":
                ph = cons.tile([V, H], BF16, tag=f"P{g}h")
                pl = cons.tile([V, H], BF16, tag=f"P{g}l")
                nc.sync.dma_start(ph[:], p_d[g][0])
                nc.sync.dma_start(pl[:], p_d[g][1])
                pw[g] = (ph, pl)
            else:
                p = cons.tile([V, H], dt_p, tag=f"P{g}")
                nc.sync.dma_start(p[:], p_d[g])
                pw[g] = p
        if mode == "bf16x3":
            wout_h = cons.tile([128, KH * V], BF16, tag="wouth")
            wout_l = cons.tile([128, KH * V], BF16, tag="woutl")
            nc.sync.dma_start(wout_h[:], wout_h_d)
            nc.sync.dma_start(wout_l[:], wout_l_d)
        else:
            wout = cons.tile([128, KH * V], dt_w, tag="wout")
            cast_dma.dma_start(wout[:], wout_d)

        # ---------- decode steps ----------
        persist = (mode == "f32r")
        ps = ctx.enter_context(tc.tile_pool(name="ps", bufs=5, space="PSUM"))
        lps = ctx.enter_context(tc.tile_pool(name="lps", bufs=1, space="PSUM"))
        tps = ctx.enter_context(tc.tile_pool(name="tps", bufs=2, space="PSUM"))
        hs = ctx.enter_context(tc.tile_pool(
            name="hsplit", bufs=(17 if persist else 9)))
        rhp = ctx.enter_context(tc.tile_pool(name="rhp", bufs=8))
        gp = ctx.enter_context(tc.tile_pool(name="gates", bufs=2))
        rhf = None if mode == "f32r" else ctx.enter_context(tc.tile_pool(
            name="rhf", bufs=(10 if mode == "f32" else 2)))
        htp = ctx.enter_context(tc.tile_pool(name="htp", bufs=2))
        sp = ctx.enter_context(tc.tile_pool(name="smax", bufs=1))

        def split_one(bh, k):
            """snapshot h[:, k, bh] as matmul rhs (pre-rounded for the MM dtype)."""
            src = h_sb[:, k, bh * NB:(bh + 1) * NB]
            if mode == "bf16x3":
                hh = hs.tile([128, NB], BF16, tag="hh")
                nc.vector.tensor_copy(hh[:], src)
                hl = hs.tile([128, NB], BF16, tag="hl")
                nc.vector.tensor_tensor(hl[:], src, hh[:], op=ALU.subtract)
                return (hh[:], hl[:])
            elif mode == "f32r":
                hr = hs.tile([128, NB], F32R, tag="hr")
                nc.vector.tensor_copy(hr[:], src)
                return hr[:]
            else:
                # snapshot: z-gate MMs must see pre-update h
                hc = hs.tile([128, NB], F32, tag="hc")
                nc.vector.tensor_copy(hc[:], src)
                return hc[:]

        def mk_hsplit(bh):
            return [split_one(bh, k) for k in range(KH)]

        def gate_mms(pt, g, m, rhs_split, t, emit_p_inline=True):
            """accumulate gate matmuls for output chunk m into psum pt.

            The K=6 prev-token matmul goes LAST so the h-part matmuls can
            start before tokenT (previous step softmax) is ready.
            """
            has_p = t > 0
            for k in range(KH):
                first = (k == 0)
                last = (k == KH - 1) and not has_p
                if mode == "bf16x3":
                    wh, wl = gw[g]
                    hh, hl = rhs_split[k]
                    lw_h = wh[:, k, m * 128:(m + 1) * 128]
                    lw_l = wl[:, k, m * 128:(m + 1) * 128]
                    nc.tensor.matmul(pt[:], lw_h, hh, start=first, stop=False)
                    nc.tensor.matmul(pt[:], lw_h, hl, start=False, stop=False)
                    nc.tensor.matmul(pt[:], lw_l, hh, start=False, stop=last)
                else:
                    w = gw[g]
                    nc.tensor.matmul(pt[:], w[:, k, m * 128:(m + 1) * 128],
                                     rhs_split[k], start=first, stop=last)
            if has_p and emit_p_inline:
                emit_p(pt, g, m)

        def emit_p(pt, g, m):
            ts = tokT[:, bh_cur * NB:(bh_cur + 1) * NB]
            if mode == "bf16x3":
                ph, pl = pw[g]
                nc.tensor.matmul(pt[:], ph[:, m * 128:(m + 1) * 128], ts,
                                 start=False, stop=False)
                nc.tensor.matmul(pt[:], pl[:, m * 128:(m + 1) * 128], ts,
                                 start=False, stop=True)
            else:
                nc.tensor.matmul(pt[:], pw[g][:, m * 128:(m + 1) * 128], ts,
                                 start=False, stop=True)

        cur_split = {bh: mk_hsplit(bh) for bh in range(BH)} if persist else {}
        for t in range(T):
            logit_sb = []
            for bh in range(BH):
                bh_cur = bh
                hsplit = cur_split[bh] if persist else mk_hsplit(bh)
                new_split = [None] * KH
                # r gates + rh products (+ splits). The K=6 prev-token matmuls
                # of the first DEFER_N groups are deferred to the phase end so
                # the PE never stalls on tokenT (previous step's softmax).
                rh_split = [None] * KH

                def r_evac_rh(pt, m):
                    bcol_r = (1 if t == 0 else 4) * KH + m
                    r_m = gp.tile([128, NB], F32, tag="r", name=f"r_{t}_{bh}_{m}")
                    nc.scalar.activation(r_m[:], pt[:], AF.Sigmoid,
                                         bias=bias_sb[:, bcol_r:bcol_r + 1])
                    if mode == "f32r":
                        rhr = rhp.tile([128, NB], F32R, tag="rhr",
                                       name=f"rhr_{t}_{bh}_{m}")
                        nc.vector.tensor_tensor(rhr[:], r_m[:],
                                                h_sb[:, m, bh * NB:(bh + 1) * NB], op=ALU.mult)
                        rh_split[m] = rhr[:]
                    else:
                        rh_m = rhf.tile([128, NB], F32, tag="rh",
                                        name=f"rh_{t}_{bh}_{m}")
                        nc.vector.tensor_tensor(rh_m[:], r_m[:],
                                                h_sb[:, m, bh * NB:(bh + 1) * NB], op=ALU.mult)
                        if mode == "bf16x3":
                            rhh = rhp.tile([128, NB], BF16, tag="rhh",
                                           name=f"rhh_{t}_{bh}_{m}")
                            nc.vector.tensor_copy(rhh[:], rh_m[:])
                            rhl = rhp.tile([128, NB], BF16, tag="rhl",
                                           name=f"rhl_{t}_{bh}_{m}")
                            nc.vector.tensor_tensor(rhl[:], rh_m[:], rhh[:], op=ALU.subtract)
                            rh_split[m] = (rhh[:], rhl[:])
                        else:
                            rh_split[m] = rh_m[:]

                DEFER_N = 0
                deferred = []
                for m in range(KH):
                    pt = ps.tile([128, NB], F32, tag="mm")
                    if m < DEFER_N:
                        gate_mms(pt, "r", m, hsplit, t, emit_p_inline=False)
                        deferred.append((pt, m))
                    else:
                        gate_mms(pt, "r", m, hsplit, t)
                        r_evac_rh(pt, m)
                for pt, m in deferred:
                    emit_p(pt, "r", m)
                    r_evac_rh(pt, m)
                # h_tilde + z + h update, per m
                for m in range(KH):
                    pt = ps.tile([128, NB], F32, tag="mm")
                    gate_mms(pt, "h", m, rh_split, t)
                    bcol_h = (2 if t == 0 else 5) * KH + m
                    ht_m = htp.tile([128, NB], F32, tag="ht")
                    nc.scalar.activation(ht_m[:], pt[:], AF.Tanh,
                                         bias=bias_sb[:, bcol_h:bcol_h + 1])
                    pt = ps.tile([128, NB], F32, tag="mm")
                    gate_mms(pt, "z", m, hsplit, t)
                    bcol_z = (0 if t == 0 else 3) * KH + m
                    z_m = gp.tile([128, NB], F32, tag="z")
                    nc.scalar.activation(z_m[:], pt[:], AF.Sigmoid,
                                         bias=bias_sb[:, bcol_z:bcol_z + 1])
                    hcur = h_sb[:, m, bh * NB:(bh + 1) * NB]
                    nc.vector.tensor_tensor(ht_m[:], ht_m[:], hcur, op=ALU.subtract)
                    nc.vector.tensor_tensor(ht_m[:], ht_m[:], z_m[:], op=ALU.mult)
                    nc.vector.tensor_tensor(hcur, hcur, ht_m[:], op=ALU.add)
                    if persist:
                        new_split[m] = split_one(bh, m)
                # logits for this bh (uses updated h)
                if persist:
                    cur_split[bh] = new_split
                    hsplit2 = new_split
                else:
                    hsplit2 = mk_hsplit(bh)
                pl_t = lps.tile([V, NB], F32, tag="lg")
                for k in range(KH):
                    first = (k == 0)
                    last = (k == KH - 1)
                    if mode == "bf16x3":
                        hh, hl = hsplit2[k]
                        nc.tensor.matmul(pl_t[:], wout_h[:, k * V:(k + 1) * V], hh,
                                         start=first, stop=False)
                        nc.tensor.matmul(pl_t[:], wout_h[:, k * V:(k + 1) * V], hl,
                                         start=False, stop=False)
                        nc.tensor.matmul(pl_t[:], wout_l[:, k * V:(k + 1) * V], hh,
                                         start=False, stop=last)
                    else:
                        nc.tensor.matmul(pl_t[:], wout[:, k * V:(k + 1) * V],
                                         hsplit2[k], start=first, stop=last)
                lsb = sp.tile([V, NB], F32, tag="lsb", bufs=2)
                nc.scalar.activation(lsb[:], pl_t[:], AF.Identity, bias=bout_sb[:])
                logit_sb.append(lsb)

            # ---- transpose logits to batch-major, (128, NCH, V)
            lbm = sp.tile([128, NCH, V], F32, tag="lbm")
            for c in range(NCH):
                bh, cc = divmod(c, NCH // 2)
                ptt = tps.tile([128, V], F32, tag="tp")
                nc.tensor.transpose(ptt[:], logit_sb[bh][:, cc * 128:(cc + 1) * 128],
                                    eye6_sb[:])
                nc.scalar.activation(lbm[:, c, :], ptt[:], AF.Copy)
            nc.sync.dma_start(log_d[t].rearrange("(c p) v -> p c v", p=128), lbm[:])

            # ---- a = logits + g_t ; softmax/argmax/alpha/token
            a_t = sp.tile([128, 48], F32, tag="a")
            a3 = a_t[:].rearrange("p (c v) -> p c v", v=V)
            g3 = g_sb[:, t * 48:(t + 1) * 48].rearrange("p (c v) -> p c v", v=V)
            nc.vector.tensor_tensor(a3, lbm[:], g3, op=ALU.add)
            m8 = sp.tile([128, NCH], F32, tag="m8")
            nc.vector.tensor_reduce(m8[:], a3, axis=mybir.AxisListType.X, op=ALU.max)
            m8b = m8[:].rearrange("p (c o) -> p c o", o=1).broadcast_to((128, NCH, V))
            eqv = sp.tile([128, 48], F32, tag="eqv")
            eq3 = eqv[:].rearrange("p (c v) -> p c v", v=V)
            nc.vector.tensor_tensor(eq3, a3, m8b, op=ALU.is_ge)
            # s = 1/sum(exp(a-m)); alpha = (1-s)+s
            sub = sp.tile([128, 48], F32, tag="sub")
            nc.vector.tensor_tensor(
                sub[:].rearrange("p (c v) -> p c v", v=V), a3, m8b, op=ALU.subtract)
            ex = sp.tile([128, 48], F32, tag="ex")
            nc.scalar.activation(ex[:], sub[:], AF.Exp)
            S8 = sp.tile([128, NCH], F32, tag="S8")
            nc.vector.tensor_reduce(S8[:], ex[:].rearrange("p (c v) -> p c v", v=V),
                                    axis=mybir.AxisListType.X, op=ALU.add)
            s8 = sp.tile([128, NCH], F32, tag="s8")
            nc.vector.reciprocal(s8[:], S8[:])
            al = sp.tile([128, NCH], F32, tag="al")
            nc.vector.tensor_scalar(al[:], s8[:], -1.0, 1.0, op0=ALU.mult, op1=ALU.add)
            nc.vector.tensor_tensor(al[:], al[:], s8[:], op=ALU.add)
            # first-argmax one-hot via iota/min
            mi = sp.tile([128, 48], F32, tag="mi")
            nc.vector.memset(mi[:], 64.0)
            nc.vector.copy_predicated(mi[:], eqv[:].bitcast(mybir.dt.uint32), iota_sb[:])
            idx8 = sp.tile([128, NCH], F32, tag="idx8")
            nc.vector.tensor_reduce(idx8[:], mi[:].rearrange("p (c v) -> p c v", v=V),
                                    axis=mybir.AxisListType.X, op=ALU.min)
            idxb = idx8[:].rearrange("p (c o) -> p c o", o=1).broadcast_to((128, NCH, V))
            tok = sp.tile([128, 48], F32, tag="tok")
            tok3 = tok[:].rearrange("p (c v) -> p c v", v=V)
            nc.vector.tensor_tensor(tok3, iota_sb[:].rearrange("p (c v) -> p c v", v=V),
                                    idxb, op=ALU.is_equal)
            alb = al[:].rearrange("p (c o) -> p c o", o=1).broadcast_to((128, NCH, V))
            nc.vector.tensor_tensor(tok3, tok3, alb, op=ALU.mult)
            # nt += rc ; masked = tok * rc ; rc *= (1 - tok[:,:,V-1])
            nc.vector.tensor_tensor(nt[:], nt[:], rc[:], op=ALU.add)
            rcb = rc[:].rearrange("p (c o) -> p c o", o=1).broadcast_to((128, NCH, V))
            nc.vector.tensor_tensor(
                msg_sb[:].rearrange("p c (t v) -> p c t v", v=V)[:, :, t, :],
                tok3, rcb, op=ALU.mult)
            tl8 = sp.tile([128, NCH], F32, tag="tl8")
            nc.vector.tensor_scalar(tl8[:], tok3[:, :, V - 1], -1.0, 1.0,
                                    op0=ALU.mult, op1=ALU.add)
            nc.vector.tensor_tensor(rc[:], rc[:], tl8[:], op=ALU.mult)
            # tokenT for next step
            if t < T - 1:
                for c in range(NCH):
                    ptt = tps.tile([V, 128], F32, tag="tp")
                    nc.tensor.transpose(ptt[:], tok[:, c * V:(c + 1) * V], eye128_sb[:])
                    nc.scalar.activation(tokT[:, c * 128:(c + 1) * 128], ptt[:], AF.Copy)

        # ---------- outputs ----------
        nc.sync.dma_start(msg_d.rearrange("(c p) w -> p c w", p=128), msg_sb[:])
        nc.sync.dma_start(ntok_d.rearrange("(c p) -> p c", p=128), nt[:])

    nc.compile()
    _BUILD_CACHE[mode] = nc
    return nc


def _prep_inputs(inputs, mode):
    f32 = np.float32
    x = np.asarray(inputs["x"], f32)
    u = np.asarray(inputs["u_noise"], f32)
    W_enc = np.asarray(inputs["W_enc"], f32)
    b_enc = np.asarray(inputs["b_enc"], f32)
    start_embed = np.asarray(inputs["start_embed"], f32)
    W_tok = np.asarray(inputs["W_tok"], f32)
    b_tok = np.asarray(inputs["b_tok"], f32)
    Wg = {g: np.asarray(inputs["W" + g], f32) for g in "zrh"}
    bg = {g: np.asarray(inputs["b" + g], f32) for g in "zrh"}
    W_out = np.asarray(inputs["W_out"], f32)
    b_out = np.asarray(inputs["b_out"], f32)

    eps = f32(1e-10)
    gum = -np.log(-np.log(u + eps) + eps)  # fp32 throughout

    rnd = _round_f32r if mode == "f32r" else (lambda a: a)
    shared = {}
    if mode == "bf16x3":
        eh, el = _split_bf16(W_enc)
        shared["Wenc_h"], shared["Wenc_l"] = eh, el
    else:
        shared["Wenc"] = rnd(W_enc)
    bias_vecs = []
    for vset in (0, 1):  # 0: step0 (start_embed), 1: steps>=1 (b_tok)
        src = start_embed if vset == 0 else b_tok
        for g in "zrh":
            top = Wg[g][:H]
            vec = (src.astype(np.float64) @ top.astype(np.float64)
                   + bg[g].astype(np.float64)).astype(f32)
            bias_vecs.append(vec)
    bias_lay = np.zeros((128, 48), f32)
    for i, vec in enumerate(bias_vecs):
        bias_lay[:, i * KH:(i + 1) * KH] = vec.reshape(KH, 128).T
    shared["bias"] = bias_lay
    for g in "zrh":
        bot = Wg[g][H:]
        P = (W_tok.astype(np.float64) @ Wg[g][:H].astype(np.float64)).astype(f32)
        if mode == "bf16x3":
            bh_, bl_ = _split_bf16(bot)
            shared[f"W{g}_h"], shared[f"W{g}_l"] = bh_, bl_
            ph_, pl_ = _split_bf16(P)
            shared[f"P{g}_h"], shared[f"P{g}_l"] = ph_, pl_
        else:
            shared[f"W{g}"] = rnd(bot)
            shared[f"P{g}"] = (P.astype(ml_dtypes.bfloat16) if mode == "f32r"
                               else P)
    wout_lay = W_out.reshape(KH, 128, V).transpose(1, 0, 2).reshape(128, KH * V)
    if mode == "bf16x3":
        wh_, wl_ = _split_bf16(wout_lay)
        shared["Wout_h"], shared["Wout_l"] = wh_, wl_
    else:
        shared["Wout"] = rnd(wout_lay)
    shared["benc"] = np.ascontiguousarray(b_enc.reshape(KH, 128).T)
    shared["bout"] = b_out.reshape(V, 1)
    shared["iota"] = np.tile(np.arange(V, dtype=f32), (128, NCH))
    shared["eye128"] = np.eye(128, dtype=f32)
    shared["eye6"] = np.eye(V, dtype=f32)

    in_maps = []
    for c in range(NC):
        m = dict(shared)
        xT = np.ascontiguousarray(x[c * BL:(c + 1) * BL].T)
        if mode == "bf16x3":
            m["xTh"], m["xTl"] = _split_bf16(xT)
        else:
            m["xT"] = rnd(xT)
        gc = gum[:, c * BL:(c + 1) * BL, :]
        m["g"] = np.ascontiguousarray(
            gc.reshape(T, NCH, 128, V).transpose(2, 0, 1, 3).reshape(128, T * 48))
        in_maps.append(m)
    return in_maps


def _run(inputs, mode=MM_MODE, trace=False):
    nc = build(mode)
    in_maps = _prep_inputs(inputs, mode)
    res = bass_utils.run_bass_kernel_spmd(nc, in_maps, list(range(NC)), trace=trace)
    msg = np.concatenate([r["msg"] for r in res.results], axis=0)
    logits = np.concatenate([r["logits"] for r in res.results], axis=1)
    ntok = np.concatenate([r["ntok"] for r in res.results], axis=0)
    return (msg, logits, ntok), res.exec_time_ns


def kernel(**inputs):
    last_err = None
    for attempt in range(3):
        try:
            out, _ = _run(inputs, MM_MODE, trace=False)
            return out
        except Exception as e:  # transient NRT device errors: retry
            last_err = e
    raise last_err


# revision 50
# speedup vs baseline: 1.0107x; 1.0107x over previous
"""Trainium2 Bass kernel for nn_AutoregressiveSender (GRU decoder + Gumbel-ST).

Self-contained: host-side prep (numpy) + SPMD Bass/Tile kernel on 8 NeuronCores.
Data parallel over batch (1024 rows/core), weights replicated.

Math notes vs reference:
- prev = token @ W_tok + b_tok is folded into the gate matmuls via
  P_g = W_tok @ Wg[:H] (6xH) and per-step bias vectors (host-precomputed),
  cutting gate contractions from 2H to H (+ a K=6 matmul).
- token == alpha * onehot(argmax(logits+g)) exactly, with
  alpha = (1-s)+s, s = 1/sum(exp(a-max)). Computed on-device.
- MM_MODE selects matmul precision:
    "f32"    exact fp32 (4 cyc/row, slowest)
    "bf16x3" hi/lo bf16 split, 3 passes (~5e-6 rel err)
    "f32r"   single-pass fp32r (~1.5e-4 rel err, fastest)
"""
import sys

for _p in ("/opt/trn_rl_repo", "/opt/pypackages"):
    if _p not in sys.path:
        sys.path.append(_p)

import numpy as np
import ml_dtypes
from contextlib import ExitStack

import concourse.tile as tile
from concourse import bacc, mybir
from concourse import bass_utils
from concourse.bass import _add_dep_helper

F32 = mybir.dt.float32
F32R = mybir.dt.float32r
BF16 = mybir.dt.bfloat16
AF = mybir.ActivationFunctionType
ALU = mybir.AluOpType

B, D, H, V, T = 8192, 2048, 1024, 6, 6
NC = 8
BL = B // NC          # 1024 rows per core
KD = D // 128         # 16 k-chunks (encoder)
KH = H // 128         # 8 k-chunks / m-chunks
BH = 2                # batch halves of 512
NB = BL // 2          # 512
NCH = BL // 128       # 8 batch chunks of 128

MM_MODE = "f32r"

_BUILD_CACHE = {}


def _split_bf16(a):
    hi = a.astype(ml_dtypes.bfloat16)
    lo = (a - hi.astype(np.float32)).astype(ml_dtypes.bfloat16)
    return hi, lo


def _round_f32r(a):
    """fp32r storage rounding: RNE to 11 mantissa bits (HW-verified)."""
    b = a.astype(np.float32).view(np.uint32).astype(np.uint64)
    shift = 12
    lsb = (b >> shift) & 1
    r = (b + ((1 << (shift - 1)) - 1 + lsb)) & ~np.uint64((1 << shift) - 1)
    return r.astype(np.uint32).view(np.float32)


def build(mode=MM_MODE):
    if mode in _BUILD_CACHE:
        return _BUILD_CACHE[mode]
    nc = bacc.Bacc("TRN2", target_bir_lowering=False, debug=False, num_devices=NC)
    dt_w = {"f32": F32, "bf16x3": BF16, "f32r": F32R}[mode]

    def din(name, shape, dt=F32):
        return nc.dram_tensor(name, shape, dt, kind="ExternalInput").ap()

    # per-core inputs
    if mode == "bf16x3":
        xTh_d = din("xTh", [D, BL], BF16)
        xTl_d = din("xTl", [D, BL], BF16)
    else:
        xT_d = din("xT", [D, BL], F32R if mode == "f32r" else F32)
    g_d = din("g", [128, T * 48])
    # replicated weights
    if mode == "bf16x3":
        wenc_h_d = din("Wenc_h", [D, H], BF16)
        wenc_l_d = din("Wenc_l", [D, H], BF16)
        gw_d = {g: (din(f"W{g}_h", [H, H], BF16), din(f"W{g}_l", [H, H], BF16))
                for g in "zrh"}
        p_d = {g: (din(f"P{g}_h", [V, H], BF16), din(f"P{g}_l", [V, H], BF16))
               for g in "zrh"}
        wout_h_d = din("Wout_h", [128, KH * V], BF16)
        wout_l_d = din("Wout_l", [128, KH * V], BF16)
    else:
        dt_in = F32R if mode == "f32r" else F32
        wenc_d = din("Wenc", [D, H], dt_in)
        gw_d = {g: din(f"W{g}", [H, H], dt_in) for g in "zrh"}
        p_d = {g: din(f"P{g}", [V, H], BF16 if mode == "f32r" else F32)
               for g in "zrh"}
        wout_d = din("Wout", [128, KH * V], dt_in)
    bias_d = din("bias", [128, 48])
    benc_d = din("benc", [128, KD // 2])
    bout_d = din("bout", [V, 1])
    iota_d = din("iota", [128, 48])
    eye128_d = din("eye128", [128, 128])
    eye6_d = din("eye6", [V, V])

    msg_d = nc.dram_tensor("msg", [BL, T * V], F32, kind="ExternalOutput").ap()
    log_d = nc.dram_tensor("logits", [T, BL, V], F32, kind="ExternalOutput").ap()
    ntok_d = nc.dram_tensor("ntok", [BL], F32, kind="ExternalOutput").ap()

    cast_dma = nc.sync  # f32r inputs are pre-rounded on host; no casting DMA
    dt_p = BF16 if mode == "f32r" else dt_w  # prev-token path dtype

    with tile.TileContext(nc) as tc, ExitStack() as ctx:
        # ---------- persistent SBUF ----------
        cons = ctx.enter_context(tc.tile_pool(name="cons", bufs=1))

        g_sb = cons.tile([128, T * 48], F32, tag="g")
        nc.sync.dma_start(g_sb[:], g_d)
        bias_sb = cons.tile([128, 48], F32, tag="bias")
        nc.sync.dma_start(bias_sb[:], bias_d)
        benc_sb = cons.tile([128, KD // 2], F32, tag="benc")
        nc.sync.dma_start(benc_sb[:], benc_d)
        bout_sb = cons.tile([V, 1], F32, tag="bout")
        nc.sync.dma_start(bout_sb[:], bout_d)
        iota_sb = cons.tile([128, 48], F32, tag="iota")
        nc.sync.dma_start(iota_sb[:], iota_d)
        eye128_sb = cons.tile([128, 128], F32, tag="eye128")
        nc.sync.dma_start(eye128_sb[:], eye128_d)
        eye6_sb = cons.tile([V, V], F32, tag="eye6")
        nc.sync.dma_start(eye6_sb[:], eye6_d)

        # state tiles
        st = ctx.enter_context(tc.tile_pool(name="state", bufs=1))
        h_sb = st.tile([128, KH, BL], F32, tag="h")
        rc = st.tile([128, NCH], F32, tag="rc")
        nt = st.tile([128, NCH], F32, tag="nt")
        msg_sb = st.tile([128, NCH, T * V], F32, tag="msg")
        tokT = st.tile([V, BL], dt_p, tag="tokT")
        nc.vector.memset(rc[:], 1.0)
        nc.vector.memset(nt[:], 0.0)

        # ---------- encoder ----------
        # W_enc is loaded into SBUF once (row-contiguous DMAs) and reused for
        # both batch halves; xT chunks stream per half.
        KHALF = KD // 2
        with tc.tile_pool(name="encps", bufs=KH, space="PSUM") as encps, \
             tc.tile_pool(name="encx", bufs=6) as encx, \
             tc.tile_pool(name="encw", bufs=2) as encw:
            if mode == "bf16x3":
                wt_h = [encw.tile([128, KHALF, H], BF16, tag="weh", name=f"weh{h}")
                        for h in range(2)]
                wt_l = [encw.tile([128, KHALF, H], BF16, tag="wel", name=f"wel{h}")
                        for h in range(2)]

                def load_wrow(half, kk):
                    k = half * KHALF + kk
                    nc.sync.dma_start(wt_h[half][:, kk, :],
                                      wenc_h_d[k * 128:(k + 1) * 128, :])
                    nc.sync.dma_start(wt_l[half][:, kk, :],
                                      wenc_l_d[k * 128:(k + 1) * 128, :])
            else:
                wts_enc = [encw.tile([128, KHALF, H], dt_w, tag="we", name=f"we{h}")
                           for h in range(2)]

                def load_wrow(half, kk):
                    k = half * KHALF + kk
                    for hh2 in range(2):
                        cast_dma.dma_start(
                            wts_enc[half][:, kk, hh2 * (H // 2):(hh2 + 1) * (H // 2)],
                            wenc_d[k * 128:(k + 1) * 128,
                                   hh2 * (H // 2):(hh2 + 1) * (H // 2)])

            for kk in range(KHALF):
                load_wrow(0, kk)
            for half in range(2):
                k0 = half * KHALF
                for bh in range(BH):
                    pts = [encps.tile([128, NB], F32, tag="enc",
                                      name=f"encp{half}_{bh}_{i}") for i in range(KH)]
                    for kk in range(KHALF):
                        k = k0 + kk
                        if mode == "bf16x3":
                            xh = encx.tile([128, NB], BF16, tag="xh")
                            nc.sync.dma_start(
                                xh[:], xTh_d[k * 128:(k + 1) * 128, bh * NB:(bh + 1) * NB])
                            xl = encx.tile([128, NB], BF16, tag="xl")
                            last_enc_dma = nc.sync.dma_start(
                                xl[:], xTl_d[k * 128:(k + 1) * 128, bh * NB:(bh + 1) * NB])
                        else:
                            xk = encx.tile([128, NB], dt_w, tag="x")
                            last_enc_dma = cast_dma.dma_start(
                                xk[:], xT_d[k * 128:(k + 1) * 128, bh * NB:(bh + 1) * NB])
                        if half == 0 and bh == 1:
                            load_wrow(1, kk)   # prefetch half1 weights in-stream
                        for m in range(KH):
                            first = (kk == 0)
                            last = (kk == KHALF - 1)
                            ms = slice(m * 128, (m + 1) * 128)
                            if mode == "bf16x3":
                                nc.tensor.matmul(pts[m][:], wt_h[half][:, kk, ms], xh[:], start=first, stop=False)
                                nc.tensor.matmul(pts[m][:], wt_h[half][:, kk, ms], xl[:], start=False, stop=False)
                                nc.tensor.matmul(pts[m][:], wt_l[half][:, kk, ms], xh[:], start=False, stop=last)
                            else:
                                nc.tensor.matmul(pts[m][:], wts_enc[half][:, kk, ms], xk[:], start=first, stop=last)
                    for m in range(KH):
                        hdst = h_sb[:, m, bh * NB:(bh + 1) * NB]
                        if half == 0:
                            # split evacs across ACT/DVE to halve the
                            # PSUM-free latency between encoder sub-phases
                            if m % 2 == 0:
                                nc.scalar.activation(hdst, pts[m][:], AF.Identity,
                                                     bias=benc_sb[:, m:m + 1])
                            else:
                                nc.vector.scalar_tensor_tensor(
                                    hdst, pts[m][:], 1.0, benc_sb[:, m:m + 1]
                                    .broadcast_to((128, NB)),
                                    op0=ALU.mult, op1=ALU.add)
                        else:
                            nc.vector.tensor_tensor(hdst, hdst, pts[m][:], op=ALU.add)

        wpool = ctx.enter_context(tc.tile_pool(name="wts", bufs=1))
        # GRU weights resident: [p, k, m] with k = contraction chunk.
        # Loaded in phase order (r, h, z) and gated behind the encoder's DMAs
        # so they don't steal HBM bandwidth from the encoder's working set.
        def after_enc(inst):
            _add_dep_helper(inst.ins, last_enc_dma.ins, sync=True,
                            reason="gate weights after encoder DMA")
            return inst

        gw = {}
        for g in "rhz":
            if mode == "bf16x3":
                wh = wpool.tile([128, KH, H], BF16, tag=f"W{g}h")
                wl = wpool.tile([128, KH, H], BF16, tag=f"W{g}l")
                after_enc(nc.sync.dma_start(
                    wh[:], gw_d[g][0].rearrange("(k p) m -> p k m", p=128)))
                after_enc(nc.sync.dma_start(
                    wl[:], gw_d[g][1].rearrange("(k p) m -> p k m", p=128)))
                gw[g] = (wh, wl)
            else:
                w = wpool.tile([128, KH, H], dt_w, tag=f"W{g}")
                after_enc(cast_dma.dma_start(
                    w[:], gw_d[g].rearrange("(k p) m -> p k m", p=128)))
                gw[g] = w
        pw = {}
        for g in "zrh":
            if mode == "bf16x3":
                ph = cons.tile([V, H], BF16, tag=f"P{g}h")
                pl = cons.tile([V, H], BF16, tag=f"P{g}l")
                nc.sync.dma_start(ph[:], p_d[g][0])
                nc.sync.dma_start(pl[:], p_d[g][1])
                pw[g] = (ph, pl)
            else:
                p = cons.tile([V, H], dt_p, tag=f"P{g}")
                nc.sync.dma_start(p[:], p_d[g])
                pw[g] = p
        if mode == "bf16x3":
            wout_h = cons.tile([128, KH * V], BF16, tag="wouth")
            wout_l = cons.tile([128, KH * V], BF16, tag="woutl")
            nc.sync.dma_start(wout_h[:], wout_h_d)
            nc.sync.dma_start(wout_l[:], wout_l_d)
        else:
            wout = cons.tile([128, KH * V], dt_w, tag="wout")
            cast_dma.dma_start(wout[:], wout_d)

        # ---------- decode steps ----------
        persist = (mode == "f32r")
        ps = ctx.enter_context(tc.tile_pool(name="ps", bufs=5, space="PSUM"))
        lps = ctx.enter_context(tc.tile_pool(name="lps", bufs=1, space="PSUM"))
        tps = ctx.enter_context(tc.tile_pool(name="tps", bufs=2, space="PSUM"))
        hs = ctx.enter_context(tc.tile_pool(
            name="hsplit", bufs=(17 if persist else 9)))
        rhp = ctx.enter_context(tc.tile_pool(name="rhp", bufs=8))
        gp = ctx.enter_context(tc.tile_pool(name="gates", bufs=2))
        rhf = None if mode == "f32r" else ctx.enter_context(tc.tile_pool(
            name="rhf", bufs=(10 if mode == "f32" else 2)))
        htp = ctx.enter_context(tc.tile_pool(name="htp", bufs=2))
        sp = ctx.enter_context(tc.tile_pool(name="smax", bufs=1))

        def split_one(bh, k):
            """snapshot h[:, k, bh] as matmul rhs (pre-rounded for the MM dtype)."""
            src = h_sb[:, k, bh * NB:(bh + 1) * NB]
            if mode == "bf16x3":
                hh = hs.tile([128, NB], BF16, tag="hh")
                nc.vector.tensor_copy(hh[:], src)
                hl = hs.tile([128, NB], BF16, tag="hl")
                nc.vector.tensor_tensor(hl[:], src, hh[:], op=ALU.subtract)
                return (hh[:], hl[:])
            elif mode == "f32r":
                hr = hs.tile([128, NB], F32R, tag="hr")
                nc.vector.tensor_copy(hr[:], src)
                return hr[:]
            else:
                # snapshot: z-gate MMs must see pre-update h
                hc = hs.tile([128, NB], F32, tag="hc")
                nc.vector.tensor_copy(hc[:], src)
                return hc[:]

        def mk_hsplit(bh):
            return [split_one(bh, k) for k in range(KH)]

        def gate_mms(pt, g, m, rhs_split, t, emit_p_inline=True):
            """accumulate gate matmuls for output chunk m into psum pt.

            The K=6 prev-token matmul goes LAST so the h-part matmuls can
            start before tokenT (previous step softmax) is ready.
            """
            has_p = t > 0
            for k in range(KH):
                first = (k == 0)
                last = (k == KH - 1) and not has_p
                if mode == "bf16x3":
                    wh, wl = gw[g]
                    hh, hl = rhs_split[k]
                    lw_h = wh[:, k, m * 128:(m + 1) * 128]
                    lw_l = wl[:, k, m * 128:(m + 1) * 128]
                    nc.tensor.matmul(pt[:], lw_h, hh, start=first, stop=False)
                    nc.tensor.matmul(pt[:], lw_h, hl, start=False, stop=False)
                    nc.tensor.matmul(pt[:], lw_l, hh, start=False, stop=last)
                else:
                    w = gw[g]
                    nc.tensor.matmul(pt[:], w[:, k, m * 128:(m + 1) * 128],
                                     rhs_split[k], start=first, stop=last)
            if has_p and emit_p_inline:
                emit_p(pt, g, m)

        def emit_p(pt, g, m):
            ts = tokT[:, bh_cur * NB:(bh_cur + 1) * NB]
            if mode == "bf16x3":
                ph, pl = pw[g]
                nc.tensor.matmul(pt[:], ph[:, m * 128:(m + 1) * 128], ts,
                                 start=False, stop=False)
                nc.tensor.matmul(pt[:], pl[:, m * 128:(m + 1) * 128], ts,
                                 start=False, stop=True)
            else:
                nc.tensor.matmul(pt[:], pw[g][:, m * 128:(m + 1) * 128], ts,
                                 start=False, stop=True)

        cur_split = {bh: mk_hsplit(bh) for bh in range(BH)} if persist else {}
        for t in range(T):
            logit_sb = []
            for bh in range(BH):
                bh_cur = bh
                hsplit = cur_split[bh] if persist else mk_hsplit(bh)
                new_split = [None] * KH
                # r gates + rh products (+ splits). The K=6 prev-token matmuls
                # of the first DEFER_N groups are deferred to the phase end so
                # the PE never stalls on tokenT (previous step's softmax).
                rh_split = [None] * KH

                def r_evac_rh(pt, m):
                    bcol_r = (1 if t == 0 else 4) * KH + m
                    r_m = gp.tile([128, NB], F32, tag="r", name=f"r_{t}_{bh}_{m}")
                    nc.scalar.activation(r_m[:], pt[:], AF.Sigmoid,
                                         bias=bias_sb[:, bcol_r:bcol_r + 1])
                    if mode == "f32r":
                        rhr = rhp.tile([128, NB], F32R, tag="rhr",
                                       name=f"rhr_{t}_{bh}_{m}")
                        nc.vector.tensor_tensor(rhr[:], r_m[:],
                                                h_sb[:, m, bh * NB:(bh + 1) * NB], op=ALU.mult)
                        rh_split[m] = rhr[:]
                    else:
                        rh_m = rhf.tile([128, NB], F32, tag="rh",
                                        name=f"rh_{t}_{bh}_{m}")
                        nc.vector.tensor_tensor(rh_m[:], r_m[:],
                                                h_sb[:, m, bh * NB:(bh + 1) * NB], op=ALU.mult)
                        if mode == "bf16x3":
                            rhh = rhp.tile([128, NB], BF16, tag="rhh",
                                           name=f"rhh_{t}_{bh}_{m}")
                            nc.vector.tensor_copy(rhh[:], rh_m[:])
                            rhl = rhp.tile([128, NB], BF16, tag="rhl",
                                           name=f"rhl_{t}_{bh}_{m}")
                            nc.vector.tensor_tensor(rhl[:], rh_m[:], rhh[:], op=ALU.subtract)
                            rh_split[m] = (rhh[:], rhl[:])
                        else:
                            rh_split[m] = rh_m[:]

                DEFER_N = 0
                deferred = []
                for m in range(KH):
                    pt = ps.tile([128, NB], F32, tag="mm")
                    if m < DEFER_N:
                        gate_mms(pt, "r", m, hsplit, t, emit_p_inline=False)
                        deferred.append((pt, m))
                    else:
                        gate_mms(pt, "r", m, hsplit, t)
                        r_evac_rh(pt, m)
                for pt, m in deferred:
                    emit_p(pt, "r", m)
                    r_evac_rh(pt, m)
                # h_tilde + z + h update, per m
                for m in range(KH):
                    pt = ps.tile([128, NB], F32, tag="mm")
                    gate_mms(pt, "h", m, rh_split, t)
                    bcol_h = (2 if t == 0 else 5) * KH + m
                    ht_m = htp.tile([128, NB], F32, tag="ht")
                    nc.scalar.activation(ht_m[:], pt[:], AF.Tanh,
                                         bias=bias_sb[:, bcol_h:bcol_h + 1])
                    pt = ps.tile([128, NB], F32, tag="mm")
                    gate_mms(pt, "z", m, hsplit, t)
                    bcol_z = (0 if t == 0 else 3) * KH + m
                    z_m = gp.tile([128, NB], F32, tag="z")
                    nc.scalar.activation(z_m[:], pt[:], AF.Sigmoid,
                                         bias=bias_sb[:, bcol_z:bcol_z + 1])
                    hcur = h_sb[:, m, bh * NB:(bh + 1) * NB]
                    nc.vector.tensor_tensor(ht_m[:], ht_m[:], hcur, op=ALU.subtract)
                    nc.vector.tensor_tensor(ht_m[:], ht_m[:], z_m[:], op=ALU.mult)
                    nc.vector.tensor_tensor(hcur, hcur, ht_m[:], op=ALU.add)
                    if persist:
                        new_split[m] = split_one(bh, m)
                # logits for this bh (uses updated h)
                if persist:
                    cur_split[bh] = new_split
                    hsplit2 = new_split
                else:
                    hsplit2 = mk_hsplit(bh)
                pl_t = lps.tile([V, NB], F32, tag="lg")
                for k in range(KH):
                    first = (k == 0)
                    last = (k == KH - 1)
                    if mode == "bf16x3":
                        hh, hl = hsplit2[k]
                        nc.tensor.matmul(pl_t[:], wout_h[:, k * V:(k + 1) * V], hh,
                                         start=first, stop=False)
                        nc.tensor.matmul(pl_t[:], wout_h[:, k * V:(k + 1) * V], hl,
                                         start=False, stop=False)
                        nc.tensor.matmul(pl_t[:], wout_l[:, k * V:(k + 1) * V], hh,
                                         start=False, stop=last)
                    else:
                        nc.tensor.matmul(pl_t[:], wout[:, k * V:(k + 1) * V],
                                         hsplit2[k], start=first, stop=last)
                lsb = sp.tile([V, NB], F32, tag="lsb", bufs=2)
                nc.scalar.activation(lsb[:], pl_t[:], AF.Identity, bias=bout_sb[:])
                logit_sb.append(lsb)

            # ---- transpose logits to batch-major, (128, NCH, V)
            lbm = sp.tile([128, NCH, V], F32, tag="lbm")
            for c in range(NCH):
                bh, cc = divmod(c, NCH // 2)
                ptt = tps.tile([128, V], F32, tag="tp")
                nc.tensor.transpose(ptt[:], logit_sb[bh][:, cc * 128:(cc + 1) * 128],
                                    eye6_sb[:])
                nc.scalar.activation(lbm[:, c, :], ptt[:], AF.Copy)
            nc.sync.dma_start(log_d[t].rearrange("(c p) v -> p c v", p=128), lbm[:])

            # ---- a = logits + g_t ; softmax/argmax/alpha/token
            a_t = sp.tile([128, 48], F32, tag="a")
            a3 = a_t[:].rearrange("p (c v) -> p c v", v=V)
            g3 = g_sb[:, t * 48:(t + 1) * 48].rearrange("p (c v) -> p c v", v=V)
            nc.vector.tensor_tensor(a3, lbm[:], g3, op=ALU.add)
            m8 = sp.tile([128, NCH], F32, tag="m8")
            nc.vector.tensor_reduce(m8[:], a3, axis=mybir.AxisListType.X, op=ALU.max)
            m8b = m8[:].rearrange("p (c o) -> p c o", o=1).broadcast_to((128, NCH, V))
            eqv = sp.tile([128, 48], F32, tag="eqv")
            eq3 = eqv[:].rearrange("p (c v) -> p c v", v=V)
            nc.vector.tensor_tensor(eq3, a3, m8b, op=ALU.is_ge)
            # s = 1/sum(exp(a-m)); alpha = (1-s)+s
            sub = sp.tile([128, 48], F32, tag="sub")
            nc.vector.tensor_tensor(
                sub[:].rearrange("p (c v) -> p c v", v=V), a3, m8b, op=ALU.subtract)
            ex = sp.tile([128, 48], F32, tag="ex")
            nc.scalar.activation(ex[:], sub[:], AF.Exp)
            S8 = sp.tile([128, NCH], F32, tag="S8")
            nc.vector.tensor_reduce(S8[:], ex[:].rearrange("p (c v) -> p c v", v=V),
                                    axis=mybir.AxisListType.X, op=ALU.add)
            s8 = sp.tile([128, NCH], F32, tag="s8")
            nc.vector.reciprocal(s8[:], S8[:])
            al = sp.tile([128, NCH], F32, tag="al")
            nc.vector.tensor_scalar(al[:], s8[:], -1.0, 1.0, op0=ALU.mult, op1=ALU.add)
            nc.vector.tensor_tensor(al[:], al[:], s8[:], op=ALU.add)
            # first-argmax one-hot via iota/min
            mi = sp.tile([128, 48], F32, tag="mi")
            nc.vector.memset(mi[:], 64.0)
            nc.vector.copy_predicated(mi[:], eqv[:].bitcast(mybir.dt.uint32), iota_sb[:])
            idx8 = sp.tile([128, NCH], F32, tag="idx8")
            nc.vector.tensor_reduce(idx8[:], mi[:].rearrange("p (c v) -> p c v", v=V),
                                    axis=mybir.AxisListType.X, op=ALU.min)
            idxb = idx8[:].rearrange("p (c o) -> p c o", o=1).broadcast_to((128, NCH, V))
            tok = sp.tile([128, 48], F32, tag="tok")
            tok3 = tok[:].rearrange("p (c v) -> p c v", v=V)
            nc.vector.tensor_tensor(tok3, iota_sb[:].rearrange("p (c v) -> p c v", v=V),
                                    idxb, op=ALU.is_equal)
            alb = al[:].rearrange("p (c o) -> p c o", o=1).broadcast_to((128, NCH, V))
            nc.vector.tensor_tensor(tok3, tok3, alb, op=ALU.mult)
            # nt += rc ; masked = tok * rc ; rc *= (1 - tok[:,:,V-1])
            nc.vector.tensor_tensor(nt[:], nt[:], rc[:], op=ALU.add)
            rcb = rc[:].rearrange("p (c o) -> p c o", o=1).broadcast_to((128, NCH, V))
            nc.vector.tensor_tensor(
                msg_sb[:].rearrange("p c (t v) -> p c t v", v=V)[:, :, t, :],
                tok3, rcb, op=ALU.mult)
            tl8 = sp.tile([128, NCH], F32, tag="tl8")
            nc.vector.tensor_scalar(tl8[:], tok3[:, :, V - 1], -1.0, 1.0,
                                    op0=ALU.mult, op1=ALU.add)
            nc.vector.tensor_tensor(rc[:], rc[:], tl8[:], op=ALU.mult)
            # tokenT for next step
            if t < T - 1:
                for c in range(NCH):
                    ptt = tps.tile([V, 128], F32, tag="tp")
                    nc.tensor.transpose(ptt[:], tok[:, c * V:(c + 1) * V], eye128_sb[:])
                    nc.scalar.activation(tokT[:, c * 128:(c + 1) * 128], ptt[:], AF.Copy)

        # ---------- outputs ----------
        nc.sync.dma_start(msg_d.rearrange("(c p) w -> p c w", p=128), msg_sb[:])
        nc.sync.dma_start(ntok_d.rearrange("(c p) -> p c", p=128), nt[:])

    nc.compile()
    _BUILD_CACHE[mode] = nc
    return nc


def _prep_inputs(inputs, mode):
    f32 = np.float32
    x = np.asarray(inputs["x"], f32)
    u = np.asarray(inputs["u_noise"], f32)
    W_enc = np.asarray(inputs["W_enc"], f32)
    b_enc = np.asarray(inputs["b_enc"], f32)
    start_embed = np.asarray(inputs["start_embed"], f32)
    W_tok = np.asarray(inputs["W_tok"], f32)
    b_tok = np.asarray(inputs["b_tok"], f32)
    Wg = {g: np.asarray(inputs["W" + g], f32) for g in "zrh"}
    bg = {g: np.asarray(inputs["b" + g], f32) for g in "zrh"}
    W_out = np.asarray(inputs["W_out"], f32)
    b_out = np.asarray(inputs["b_out"], f32)

    eps = f32(1e-10)
    gum = -np.log(-np.log(u + eps) + eps)  # fp32 throughout

    rnd = _round_f32r if mode == "f32r" else (lambda a: a)
    shared = {}
    if mode == "bf16x3":
        eh, el = _split_bf16(W_enc)
        shared["Wenc_h"], shared["Wenc_l"] = eh, el
    else:
        shared["Wenc"] = rnd(W_enc)
    bias_vecs = []
    for vset in (0, 1):  # 0: step0 (start_embed), 1: steps>=1 (b_tok)
        src = start_embed if vset == 0 else b_tok
        for g in "zrh":
            top = Wg[g][:H]
            vec = (src.astype(np.float64) @ top.astype(np.float64)
                   + bg[g].astype(np.float64)).astype(f32)
            bias_vecs.append(vec)
    bias_lay = np.zeros((128, 48), f32)
    for i, vec in enumerate(bias_vecs):
        bias_lay[:, i * KH:(i + 1) * KH] = vec.reshape(KH, 128).T
    shared["bias"] = bias_lay
    for g in "zrh":
        bot = Wg[g][H:]
        P = (W_tok.astype(np.float64) @ Wg[g][:H].astype(np.float64)).astype(f32)
        if mode == "bf16x3":
            bh_, bl_ = _split_bf16(bot)
            shared[f"W{g}_h"], shared[f"W{g}_l"] = bh_, bl_
            ph_, pl_ = _split_bf16(P)
            shared[f"P{g}_h"], shared[f"P{g}_l"] = ph_, pl_
        else:
            shared[f"W{g}"] = rnd(bot)
            shared[f"P{g}"] = (P.astype(ml_dtypes.bfloat16) if mode == "f32r"
                               else P)
    wout_lay = W_out.reshape(KH, 128, V).transpose(1, 0, 2).reshape(128, KH * V)
    if mode == "bf16x3":
        wh_, wl_ = _split_bf16(wout_lay)
        shared["Wout_h"], shared["Wout_l"] = wh_, wl_
    else:
        shared["Wout"] = rnd(wout_lay)
    shared["benc"] = np.ascontiguousarray(b_enc.reshape(KH, 128).T)
    shared["bout"] = b_out.reshape(V, 1)
    shared["iota"] = np.tile(np.arange(V, dtype=f32), (128, NCH))
    shared["eye128"] = np.eye(128, dtype=f32)
    shared["eye6"] = np.eye(V, dtype=f32)

    in_maps = []
    for c in range(NC):
        m = dict(shared)
        xT = np.ascontiguousarray(x[c * BL:(c + 1) * BL].T)
        if mode == "bf16x3":
            m["xTh"], m["xTl"] = _split_bf16(xT)
        else:
            m["xT"] = rnd(xT)
        gc = gum[:, c * BL:(c + 1) * BL, :]
        m["g"] = np.ascontiguousarray(
            gc.reshape(T, NCH, 128, V).transpose(2, 0, 1, 3).reshape(128, T * 48))
        in_maps.append(m)
    return in_maps


def _run(inputs, mode=MM_MODE, trace=False):
    nc = build(mode)
    in_maps = _prep_inputs(inputs, mode)
    res = bass_utils.run_bass_kernel_spmd(nc, in_maps, list(range(NC)), trace=trace)
    msg = np.concatenate([r["msg"] for r in res.results], axis=0)
    logits = np.concatenate([r["logits"] for r in res.results], axis=1)
    ntok = np.concatenate([r["ntok"] for r in res.results], axis=0)
    return (msg, logits, ntok), res.exec_time_ns


def kernel(**inputs):
    last_err = None
    for attempt in range(3):
        try:
            out, _ = _run(inputs, MM_MODE, trace=False)
            return out
        except Exception as e:  # transient NRT device errors: retry
            last_err = e
    raise last_err


# revision 51
# speedup vs baseline: 1.0346x; 1.0237x over previous
"""Trainium2 Bass kernel for nn_AutoregressiveSender (GRU decoder + Gumbel-ST).

Self-contained: host-side prep (numpy) + SPMD Bass/Tile kernel on 8 NeuronCores.
Data parallel over batch (1024 rows/core), weights replicated.

Math notes vs reference:
- prev = token @ W_tok + b_tok is folded into the gate matmuls via
  P_g = W_tok @ Wg[:H] (6xH) and per-step bias vectors (host-precomputed),
  cutting gate contractions from 2H to H (+ a K=6 matmul).
- token == alpha * onehot(argmax(logits+g)) exactly, with
  alpha = (1-s)+s, s = 1/sum(exp(a-max)). Computed on-device.
- MM_MODE selects matmul precision:
    "f32"    exact fp32 (4 cyc/row, slowest)
    "bf16x3" hi/lo bf16 split, 3 passes (~5e-6 rel err)
    "f32r"   single-pass fp32r (~1.5e-4 rel err, fastest)
"""
import sys

for _p in ("/opt/trn_rl_repo", "/opt/pypackages"):
    if _p not in sys.path:
        sys.path.append(_p)

import numpy as np
import ml_dtypes
from contextlib import ExitStack

import concourse.tile as tile
from concourse import bacc, mybir
from concourse import bass_utils
from concourse.bass import _add_dep_helper

F32 = mybir.dt.float32
F32R = mybir.dt.float32r
BF16 = mybir.dt.bfloat16
AF = mybir.ActivationFunctionType
ALU = mybir.AluOpType

B, D, H, V, T = 8192, 2048, 1024, 6, 6
NC = 8
BL = B // NC          # 1024 rows per core
KD = D // 128         # 16 k-chunks (encoder)
KH = H // 128         # 8 k-chunks / m-chunks
BH = 2                # batch halves of 512
NB = BL // 2          # 512
NCH = BL // 128       # 8 batch chunks of 128

MM_MODE = "f32r"

_BUILD_CACHE = {}


def _split_bf16(a):
    hi = a.astype(ml_dtypes.bfloat16)
    lo = (a - hi.astype(np.float32)).astype(ml_dtypes.bfloat16)
    return hi, lo


def _round_f32r(a):
    """fp32r storage rounding: RNE to 11 mantissa bits (HW-verified)."""
    b = a.astype(np.float32).view(np.uint32).astype(np.uint64)
    shift = 12
    lsb = (b >> shift) & 1
    r = (b + ((1 << (shift - 1)) - 1 + lsb)) & ~np.uint64((1 << shift) - 1)
    return r.astype(np.uint32).view(np.float32)


def build(mode=MM_MODE):
    if mode in _BUILD_CACHE:
        return _BUILD_CACHE[mode]
    nc = bacc.Bacc("TRN2", target_bir_lowering=False, debug=False, num_devices=NC)
    dt_w = {"f32": F32, "bf16x3": BF16, "f32r": F32R}[mode]

    def din(name, shape, dt=F32):
        return nc.dram_tensor(name, shape, dt, kind="ExternalInput").ap()

    # per-core inputs
    if mode == "bf16x3":
        xTh_d = din("xTh", [D, BL], BF16)
        xTl_d = din("xTl", [D, BL], BF16)
    else:
        xT_d = din("xT", [D, BL], F32R if mode == "f32r" else F32)
    g_d = din("g", [128, T * 48])
    # replicated weights
    if mode == "bf16x3":
        wenc_h_d = din("Wenc_h", [D, H], BF16)
        wenc_l_d = din("Wenc_l", [D, H], BF16)
        gw_d = {g: (din(f"W{g}_h", [H, H], BF16), din(f"W{g}_l", [H, H], BF16))
                for g in "zrh"}
        p_d = {g: (din(f"P{g}_h", [V, H], BF16), din(f"P{g}_l", [V, H], BF16))
               for g in "zrh"}
        wout_h_d = din("Wout_h", [128, KH * V], BF16)
        wout_l_d = din("Wout_l", [128, KH * V], BF16)
    else:
        dt_in = F32R if mode == "f32r" else F32
        wenc_d = din("Wenc", [D, H], dt_in)
        gw_d = {g: din(f"W{g}", [H, H], dt_in) for g in "zrh"}
        p_d = {g: din(f"P{g}", [V, H], BF16 if mode == "f32r" else F32)
               for g in "zrh"}
        wout_d = din("Wout", [128, KH * V], dt_in)
    bias_d = din("bias", [128, 48])
    benc_d = din("benc", [128, KD // 2])
    bout_d = din("bout", [V, 1])
    iota_d = din("iota", [128, 48])
    eye128_d = din("eye128", [128, 128])
    eye6_d = din("eye6", [V, V])

    msg_d = nc.dram_tensor("msg", [BL, T * V], F32, kind="ExternalOutput").ap()
    log_d = nc.dram_tensor("logits", [T, BL, V], F32, kind="ExternalOutput").ap()
    ntok_d = nc.dram_tensor("ntok", [BL], F32, kind="ExternalOutput").ap()

    cast_dma = nc.sync  # f32r inputs are pre-rounded on host; no casting DMA
    dt_p = BF16 if mode == "f32r" else dt_w  # prev-token path dtype

    with tile.TileContext(nc) as tc, ExitStack() as ctx:
        # ---------- persistent SBUF ----------
        cons = ctx.enter_context(tc.tile_pool(name="cons", bufs=1))

        g_sb = cons.tile([128, T * 48], F32, tag="g")
        nc.sync.dma_start(g_sb[:], g_d)
        bias_sb = cons.tile([128, 48], F32, tag="bias")
        nc.sync.dma_start(bias_sb[:], bias_d)
        benc_sb = cons.tile([128, KD // 2], F32, tag="benc")
        nc.sync.dma_start(benc_sb[:], benc_d)
        bout_sb = cons.tile([V, 1], F32, tag="bout")
        nc.sync.dma_start(bout_sb[:], bout_d)
        iota_sb = cons.tile([128, 48], F32, tag="iota")
        nc.sync.dma_start(iota_sb[:], iota_d)
        eye128_sb = cons.tile([128, 128], F32, tag="eye128")
        nc.sync.dma_start(eye128_sb[:], eye128_d)
        eye6_sb = cons.tile([V, V], F32, tag="eye6")
        nc.sync.dma_start(eye6_sb[:], eye6_d)

        # state tiles
        st = ctx.enter_context(tc.tile_pool(name="state", bufs=1))
        h_sb = st.tile([128, KH, BL], F32, tag="h")
        rc = st.tile([128, NCH], F32, tag="rc")
        nt = st.tile([128, NCH], F32, tag="nt")
        msg_sb = st.tile([128, NCH, T * V], F32, tag="msg")
        tokT = st.tile([V, BL], dt_p, tag="tokT")
        nc.vector.memset(rc[:], 1.0)
        nc.vector.memset(nt[:], 0.0)

        # ---------- encoder ----------
        # W_enc is loaded into SBUF once (row-contiguous DMAs) and reused for
        # both batch halves; xT chunks stream per half.
        KHALF = KD // 2
        with tc.tile_pool(name="encps", bufs=KH, space="PSUM") as encps, \
             tc.tile_pool(name="encx", bufs=6) as encx, \
             tc.tile_pool(name="encw", bufs=2) as encw:
            if mode == "bf16x3":
                wt_h = [encw.tile([128, KHALF, H], BF16, tag="weh", name=f"weh{h}")
                        for h in range(2)]
                wt_l = [encw.tile([128, KHALF, H], BF16, tag="wel", name=f"wel{h}")
                        for h in range(2)]

                def load_wrow(half, kk):
                    k = half * KHALF + kk
                    nc.sync.dma_start(wt_h[half][:, kk, :],
                                      wenc_h_d[k * 128:(k + 1) * 128, :])
                    nc.sync.dma_start(wt_l[half][:, kk, :],
                                      wenc_l_d[k * 128:(k + 1) * 128, :])
            else:
                wts_enc = [encw.tile([128, KHALF, H], dt_w, tag="we", name=f"we{h}")
                           for h in range(2)]

                def load_wrow(half, kk):
                    k = half * KHALF + kk
                    for hh2 in range(2):
                        cast_dma.dma_start(
                            wts_enc[half][:, kk, hh2 * (H // 2):(hh2 + 1) * (H // 2)],
                            wenc_d[k * 128:(k + 1) * 128,
                                   hh2 * (H // 2):(hh2 + 1) * (H // 2)])

            for kk in range(KHALF):
                load_wrow(0, kk)
            for half in range(2):
                k0 = half * KHALF
                for bh in range(BH):
                    pts = [encps.tile([128, NB], F32, tag="enc",
                                      name=f"encp{half}_{bh}_{i}") for i in range(KH)]
                    for kk in range(KHALF):
                        k = k0 + kk
                        if mode == "bf16x3":
                            xh = encx.tile([128, NB], BF16, tag="xh")
                            nc.sync.dma_start(
                                xh[:], xTh_d[k * 128:(k + 1) * 128, bh * NB:(bh + 1) * NB])
                            xl = encx.tile([128, NB], BF16, tag="xl")
                            last_enc_dma = nc.sync.dma_start(
                                xl[:], xTl_d[k * 128:(k + 1) * 128, bh * NB:(bh + 1) * NB])
                        else:
                            xk = encx.tile([128, NB], dt_w, tag="x")
                            last_enc_dma = cast_dma.dma_start(
                                xk[:], xT_d[k * 128:(k + 1) * 128, bh * NB:(bh + 1) * NB])
                        if half == 0 and bh == 1:
                            load_wrow(1, kk)   # prefetch half1 weights in-stream
                        for m in range(KH):
                            first = (kk == 0)
                            last = (kk == KHALF - 1)
                            ms = slice(m * 128, (m + 1) * 128)
                            if mode == "bf16x3":
                                nc.tensor.matmul(pts[m][:], wt_h[half][:, kk, ms], xh[:], start=first, stop=False)
                                nc.tensor.matmul(pts[m][:], wt_h[half][:, kk, ms], xl[:], start=False, stop=False)
                                nc.tensor.matmul(pts[m][:], wt_l[half][:, kk, ms], xh[:], start=False, stop=last)
                            else:
                                nc.tensor.matmul(pts[m][:], wts_enc[half][:, kk, ms], xk[:], start=first, stop=last)
                    for m in range(KH):
                        hdst = h_sb[:, m, bh * NB:(bh + 1) * NB]
                        if half == 0:
                            # split evacs across ACT/DVE to halve the
                            # PSUM-free latency between encoder sub-phases
                            if m % 2 == 0:
                                nc.scalar.activation(hdst, pts[m][:], AF.Identity,
                                                     bias=benc_sb[:, m:m + 1])
                            else:
                                nc.vector.scalar_tensor_tensor(
                                    hdst, pts[m][:], 1.0, benc_sb[:, m:m + 1]
                                    .broadcast_to((128, NB)),
                                    op0=ALU.mult, op1=ALU.add)
                        else:
                            nc.vector.tensor_tensor(hdst, hdst, pts[m][:], op=ALU.add)

        wpool = ctx.enter_context(tc.tile_pool(name="wts", bufs=1))
        # GRU weights resident: [p, k, m] with k = contraction chunk.
        # Loaded in phase order (r, h, z) and gated behind the encoder's DMAs
        # so they don't steal HBM bandwidth from the encoder's working set.
        def after_enc(inst):
            _add_dep_helper(inst.ins, last_enc_dma.ins, sync=True,
                            reason="gate weights after encoder DMA")
            return inst

        gw = {}
        for g in "rhz":
            if mode == "bf16x3":
                wh = wpool.tile([128, KH, H], BF16, tag=f"W{g}h")
                wl = wpool.tile([128, KH, H], BF16, tag=f"W{g}l")
                after_enc(nc.sync.dma_start(
                    wh[:], gw_d[g][0].rearrange("(k p) m -> p k m", p=128)))
                after_enc(nc.sync.dma_start(
                    wl[:], gw_d[g][1].rearrange("(k p) m -> p k m", p=128)))
                gw[g] = (wh, wl)
            else:
                w = wpool.tile([128, KH, H], dt_w, tag=f"W{g}")
                after_enc(cast_dma.dma_start(
                    w[:], gw_d[g].rearrange("(k p) m -> p k m", p=128)))
                gw[g] = w
        pw = {}
        for g in "zrh":
            if mode == "bf16x3":
                ph = cons.tile([V, H], BF16, tag=f"P{g}h")
                pl = cons.tile([V, H], BF16, tag=f"P{g}l")
                nc.sync.dma_start(ph[:], p_d[g][0])
                nc.sync.dma_start(pl[:], p_d[g][1])
                pw[g] = (ph, pl)
            else:
                p = cons.tile([V, H], dt_p, tag=f"P{g}")
                nc.sync.dma_start(p[:], p_d[g])
                pw[g] = p
        if mode == "bf16x3":
            wout_h = cons.tile([128, KH * V], BF16, tag="wouth")
            wout_l = cons.tile([128, KH * V], BF16, tag="woutl")
            nc.sync.dma_start(wout_h[:], wout_h_d)
            nc.sync.dma_start(wout_l[:], wout_l_d)
        else:
            wout = cons.tile([128, KH * V], dt_w, tag="wout")
            cast_dma.dma_start(wout[:], wout_d)

        # ---------- decode steps ----------
        persist = (mode == "f32r")
        ps = ctx.enter_context(tc.tile_pool(name="ps", bufs=5, space="PSUM"))
        lps = ctx.enter_context(tc.tile_pool(name="lps", bufs=1, space="PSUM"))
        tps = ctx.enter_context(tc.tile_pool(name="tps", bufs=2, space="PSUM"))
        hs = ctx.enter_context(tc.tile_pool(
            name="hsplit", bufs=(17 if persist else 9)))
        rhp = ctx.enter_context(tc.tile_pool(name="rhp", bufs=8))
        gp = ctx.enter_context(tc.tile_pool(name="gates", bufs=2))
        rhf = None if mode == "f32r" else ctx.enter_context(tc.tile_pool(
            name="rhf", bufs=(10 if mode == "f32" else 2)))
        htp = ctx.enter_context(tc.tile_pool(name="htp", bufs=2))
        sp = ctx.enter_context(tc.tile_pool(name="smax", bufs=1))

        def split_one(bh, k):
            """snapshot h[:, k, bh] as matmul rhs (pre-rounded for the MM dtype)."""
            src = h_sb[:, k, bh * NB:(bh + 1) * NB]
            if mode == "bf16x3":
                hh = hs.tile([128, NB], BF16, tag="hh")
                nc.vector.tensor_copy(hh[:], src)
                hl = hs.tile([128, NB], BF16, tag="hl")
                nc.vector.tensor_tensor(hl[:], src, hh[:], op=ALU.subtract)
                return (hh[:], hl[:])
            elif mode == "f32r":
                hr = hs.tile([128, NB], F32R, tag="hr")
                nc.vector.tensor_copy(hr[:], src)
                return hr[:]
            else:
                # snapshot: z-gate MMs must see pre-update h
                hc = hs.tile([128, NB], F32, tag="hc")
                nc.vector.tensor_copy(hc[:], src)
                return hc[:]

        def mk_hsplit(bh):
            return [split_one(bh, k) for k in range(KH)]

        def gate_mms(pt, g, m, rhs_split, t, emit_p_inline=True):
            """accumulate gate matmuls for output chunk m into psum pt.

            The K=6 prev-token matmul goes LAST so the h-part matmuls can
            start before tokenT (previous step softmax) is ready.
            """
            has_p = t > 0
            for k in range(KH):
                first = (k == 0)
                last = (k == KH - 1) and not has_p
                if mode == "bf16x3":
                    wh, wl = gw[g]
                    hh, hl = rhs_split[k]
                    lw_h = wh[:, k, m * 128:(m + 1) * 128]
                    lw_l = wl[:, k, m * 128:(m + 1) * 128]
                    nc.tensor.matmul(pt[:], lw_h, hh, start=first, stop=False)
                    nc.tensor.matmul(pt[:], lw_h, hl, start=False, stop=False)
                    nc.tensor.matmul(pt[:], lw_l, hh, start=False, stop=last)
                else:
                    w = gw[g]
                    nc.tensor.matmul(pt[:], w[:, k, m * 128:(m + 1) * 128],
                                     rhs_split[k], start=first, stop=last)
            if has_p and emit_p_inline:
                emit_p(pt, g, m)

        def emit_p(pt, g, m):
            ts = tokT[:, bh_cur * NB:(bh_cur + 1) * NB]
            if mode == "bf16x3":
                ph, pl = pw[g]
                nc.tensor.matmul(pt[:], ph[:, m * 128:(m + 1) * 128], ts,
                                 start=False, stop=False)
                nc.tensor.matmul(pt[:], pl[:, m * 128:(m + 1) * 128], ts,
                                 start=False, stop=True)
            else:
                nc.tensor.matmul(pt[:], pw[g][:, m * 128:(m + 1) * 128], ts,
                                 start=False, stop=True)

        cur_split = {bh: mk_hsplit(bh) for bh in range(BH)} if persist else {}
        for t in range(T):
            logit_sb = []
            for bh in range(BH):
                bh_cur = bh
                hsplit = cur_split[bh] if persist else mk_hsplit(bh)
                new_split = [None] * KH
                # r gates + rh products (+ splits). The K=6 prev-token matmuls
                # of the first DEFER_N groups are deferred to the phase end so
                # the PE never stalls on tokenT (previous step's softmax).
                rh_split = [None] * KH

                def r_evac_rh(pt, m):
                    bcol_r = (1 if t == 0 else 4) * KH + m
                    r_m = gp.tile([128, NB], F32, tag="r", name=f"r_{t}_{bh}_{m}")
                    nc.scalar.activation(r_m[:], pt[:], AF.Sigmoid,
                                         bias=bias_sb[:, bcol_r:bcol_r + 1])
                    if mode == "f32r":
                        rhr = rhp.tile([128, NB], F32R, tag="rhr",
                                       name=f"rhr_{t}_{bh}_{m}")
                        nc.vector.tensor_tensor(rhr[:], r_m[:],
                                                h_sb[:, m, bh * NB:(bh + 1) * NB], op=ALU.mult)
                        rh_split[m] = rhr[:]
                    else:
                        rh_m = rhf.tile([128, NB], F32, tag="rh",
                                        name=f"rh_{t}_{bh}_{m}")
                        nc.vector.tensor_tensor(rh_m[:], r_m[:],
                                                h_sb[:, m, bh * NB:(bh + 1) * NB], op=ALU.mult)
                        if mode == "bf16x3":
                            rhh = rhp.tile([128, NB], BF16, tag="rhh",
                                           name=f"rhh_{t}_{bh}_{m}")
                            nc.vector.tensor_copy(rhh[:], rh_m[:])
                            rhl = rhp.tile([128, NB], BF16, tag="rhl",
                                           name=f"rhl_{t}_{bh}_{m}")
                            nc.vector.tensor_tensor(rhl[:], rh_m[:], rhh[:], op=ALU.subtract)
                            rh_split[m] = (rhh[:], rhl[:])
                        else:
                            rh_split[m] = rh_m[:]

                DEFER_N = 0
                deferred = []
                for m in range(KH):
                    pt = ps.tile([128, NB], F32, tag="mm")
                    if m < DEFER_N:
                        gate_mms(pt, "r", m, hsplit, t, emit_p_inline=False)
                        deferred.append((pt, m))
                    else:
                        gate_mms(pt, "r", m, hsplit, t)
                        r_evac_rh(pt, m)
                for pt, m in deferred:
                    emit_p(pt, "r", m)
                    r_evac_rh(pt, m)
                # h_tilde + z + h update, per m. The two bf16 K=6
                # prev-token matmuls are emitted adjacently so the second's
                # weight load pipelines (no fp32r->bf16 switch between them).
                for m in range(KH):
                    pt_h = ps.tile([128, NB], F32, tag="mm")
                    gate_mms(pt_h, "h", m, rh_split, t, emit_p_inline=False)
                    pt = ps.tile([128, NB], F32, tag="mm")
                    gate_mms(pt, "z", m, hsplit, t, emit_p_inline=False)
                    if t > 0:
                        emit_p(pt_h, "h", m)
                        emit_p(pt, "z", m)
                    bcol_h = (2 if t == 0 else 5) * KH + m
                    ht_m = htp.tile([128, NB], F32, tag="ht")
                    nc.scalar.activation(ht_m[:], pt_h[:], AF.Tanh,
                                         bias=bias_sb[:, bcol_h:bcol_h + 1])
                    bcol_z = (0 if t == 0 else 3) * KH + m
                    z_m = gp.tile([128, NB], F32, tag="z")
                    nc.scalar.activation(z_m[:], pt[:], AF.Sigmoid,
                                         bias=bias_sb[:, bcol_z:bcol_z + 1])
                    hcur = h_sb[:, m, bh * NB:(bh + 1) * NB]
                    nc.vector.tensor_tensor(ht_m[:], ht_m[:], hcur, op=ALU.subtract)
                    nc.vector.tensor_tensor(ht_m[:], ht_m[:], z_m[:], op=ALU.mult)
                    nc.vector.tensor_tensor(hcur, hcur, ht_m[:], op=ALU.add)
                    if persist:
                        new_split[m] = split_one(bh, m)
                # logits for this bh (uses updated h)
                if persist:
                    cur_split[bh] = new_split
                    hsplit2 = new_split
                else:
                    hsplit2 = mk_hsplit(bh)
                pl_t = lps.tile([V, NB], F32, tag="lg")
                for k in range(KH):
                    first = (k == 0)
                    last = (k == KH - 1)
                    if mode == "bf16x3":
                        hh, hl = hsplit2[k]
                        nc.tensor.matmul(pl_t[:], wout_h[:, k * V:(k + 1) * V], hh,
                                         start=first, stop=False)
                        nc.tensor.matmul(pl_t[:], wout_h[:, k * V:(k + 1) * V], hl,
                                         start=False, stop=False)
                        nc.tensor.matmul(pl_t[:], wout_l[:, k * V:(k + 1) * V], hh,
                                         start=False, stop=last)
                    else:
                        nc.tensor.matmul(pl_t[:], wout[:, k * V:(k + 1) * V],
                                         hsplit2[k], start=first, stop=last)
                lsb = sp.tile([V, NB], F32, tag="lsb", bufs=2)
                nc.scalar.activation(lsb[:], pl_t[:], AF.Identity, bias=bout_sb[:])
                logit_sb.append(lsb)

            # ---- transpose logits to batch-major, (128, NCH, V)
            lbm = sp.tile([128, NCH, V], F32, tag="lbm")
            for c in range(NCH):
                bh, cc = divmod(c, NCH // 2)
                ptt = tps.tile([128, V], F32, tag="tp")
                nc.tensor.transpose(ptt[:], logit_sb[bh][:, cc * 128:(cc + 1) * 128],
                                    eye6_sb[:])
                nc.scalar.activation(lbm[:, c, :], ptt[:], AF.Copy)
            nc.sync.dma_start(log_d[t].rearrange("(c p) v -> p c v", p=128), lbm[:])

            # ---- a = logits + g_t ; softmax/argmax/alpha/token
            a_t = sp.tile([128, 48], F32, tag="a")
            a3 = a_t[:].rearrange("p (c v) -> p c v", v=V)
            g3 = g_sb[:, t * 48:(t + 1) * 48].rearrange("p (c v) -> p c v", v=V)
            nc.vector.tensor_tensor(a3, lbm[:], g3, op=ALU.add)
            m8 = sp.tile([128, NCH], F32, tag="m8")
            nc.vector.tensor_reduce(m8[:], a3, axis=mybir.AxisListType.X, op=ALU.max)
            m8b = m8[:].rearrange("p (c o) -> p c o", o=1).broadcast_to((128, NCH, V))
            eqv = sp.tile([128, 48], F32, tag="eqv")
            eq3 = eqv[:].rearrange("p (c v) -> p c v", v=V)
            nc.vector.tensor_tensor(eq3, a3, m8b, op=ALU.is_ge)
            # s = 1/sum(exp(a-m)); alpha = (1-s)+s
            sub = sp.tile([128, 48], F32, tag="sub")
            nc.vector.tensor_tensor(
                sub[:].rearrange("p (c v) -> p c v", v=V), a3, m8b, op=ALU.subtract)
            ex = sp.tile([128, 48], F32, tag="ex")
            nc.scalar.activation(ex[:], sub[:], AF.Exp)
            S8 = sp.tile([128, NCH], F32, tag="S8")
            nc.vector.tensor_reduce(S8[:], ex[:].rearrange("p (c v) -> p c v", v=V),
                                    axis=mybir.AxisListType.X, op=ALU.add)
            s8 = sp.tile([128, NCH], F32, tag="s8")
            nc.vector.reciprocal(s8[:], S8[:])
            al = sp.tile([128, NCH], F32, tag="al")
            nc.vector.tensor_scalar(al[:], s8[:], -1.0, 1.0, op0=ALU.mult, op1=ALU.add)
            nc.vector.tensor_tensor(al[:], al[:], s8[:], op=ALU.add)
            # first-argmax one-hot via iota/min
            mi = sp.tile([128, 48], F32, tag="mi")
            nc.vector.memset(mi[:], 64.0)
            nc.vector.copy_predicated(mi[:], eqv[:].bitcast(mybir.dt.uint32), iota_sb[:])
            idx8 = sp.tile([128, NCH], F32, tag="idx8")
            nc.vector.tensor_reduce(idx8[:], mi[:].rearrange("p (c v) -> p c v", v=V),
                                    axis=mybir.AxisListType.X, op=ALU.min)
            idxb = idx8[:].rearrange("p (c o) -> p c o", o=1).broadcast_to((128, NCH, V))
            tok = sp.tile([128, 48], F32, tag="tok")
            tok3 = tok[:].rearrange("p (c v) -> p c v", v=V)
            nc.vector.tensor_tensor(tok3, iota_sb[:].rearrange("p (c v) -> p c v", v=V),
                                    idxb, op=ALU.is_equal)
            alb = al[:].rearrange("p (c o) -> p c o", o=1).broadcast_to((128, NCH, V))
            nc.vector.tensor_tensor(tok3, tok3, alb, op=ALU.mult)
            # nt += rc ; masked = tok * rc ; rc *= (1 - tok[:,:,V-1])
            nc.vector.tensor_tensor(nt[:], nt[:], rc[:], op=ALU.add)
            rcb = rc[:].rearrange("p (c o) -> p c o", o=1).broadcast_to((128, NCH, V))
            nc.vector.tensor_tensor(
                msg_sb[:].rearrange("p c (t v) -> p c t v", v=V)[:, :, t, :],
                tok3, rcb, op=ALU.mult)
            tl8 = sp.tile([128, NCH], F32, tag="tl8")
            nc.vector.tensor_scalar(tl8[:], tok3[:, :, V - 1], -1.0, 1.0,
                                    op0=ALU.mult, op1=ALU.add)
            nc.vector.tensor_tensor(rc[:], rc[:], tl8[:], op=ALU.mult)
            # tokenT for next step
            if t < T - 1:
                for c in range(NCH):
                    ptt = tps.tile([V, 128], F32, tag="tp")
                    nc.tensor.transpose(ptt[:], tok[:, c * V:(c + 1) * V], eye128_sb[:])
                    nc.scalar.activation(tokT[:, c * 128:(c + 1) * 128], ptt[:], AF.Copy)

        # ---------- outputs ----------
        nc.sync.dma_start(msg_d.rearrange("(c p) w -> p c w", p=128), msg_sb[:])
        nc.sync.dma_start(ntok_d.rearrange("(c p) -> p c", p=128), nt[:])

    nc.compile()
    _BUILD_CACHE[mode] = nc
    return nc


def _prep_inputs(inputs, mode):
    f32 = np.float32
    x = np.asarray(inputs["x"], f32)
    u = np.asarray(inputs["u_noise"], f32)
    W_enc = np.asarray(inputs["W_enc"], f32)
    b_enc = np.asarray(inputs["b_enc"], f32)
    start_embed = np.asarray(inputs["start_embed"], f32)
    W_tok = np.asarray(inputs["W_tok"], f32)
    b_tok = np.asarray(inputs["b_tok"], f32)
    Wg = {g: np.asarray(inputs["W" + g], f32) for g in "zrh"}
    bg = {g: np.asarray(inputs["b" + g], f32) for g in "zrh"}
    W_out = np.asarray(inputs["W_out"], f32)
    b_out = np.asarray(inputs["b_out"], f32)

    eps = f32(1e-10)
    gum = -np.log(-np.log(u + eps) + eps)  # fp32 throughout

    rnd = _round_f32r if mode == "f32r" else (lambda a: a)
    shared = {}
    if mode == "bf16x3":
        eh, el = _split_bf16(W_enc)
        shared["Wenc_h"], shared["Wenc_l"] = eh, el
    else:
        shared["Wenc"] = rnd(W_enc)
    bias_vecs = []
    for vset in (0, 1):  # 0: step0 (start_embed), 1: steps>=1 (b_tok)
        src = start_embed if vset == 0 else b_tok
        for g in "zrh":
            top = Wg[g][:H]
            vec = (src.astype(np.float64) @ top.astype(np.float64)
                   + bg[g].astype(np.float64)).astype(f32)
            bias_vecs.append(vec)
    bias_lay = np.zeros((128, 48), f32)
    for i, vec in enumerate(bias_vecs):
        bias_lay[:, i * KH:(i + 1) * KH] = vec.reshape(KH, 128).T
    shared["bias"] = bias_lay
    for g in "zrh":
        bot = Wg[g][H:]
        P = (W_tok.astype(np.float64) @ Wg[g][:H].astype(np.float64)).astype(f32)
        if mode == "bf16x3":
            bh_, bl_ = _split_bf16(bot)
            shared[f"W{g}_h"], shared[f"W{g}_l"] = bh_, bl_
            ph_, pl_ = _split_bf16(P)
            shared[f"P{g}_h"], shared[f"P{g}_l"] = ph_, pl_
        else:
            shared[f"W{g}"] = rnd(bot)
            shared[f"P{g}"] = (P.astype(ml_dtypes.bfloat16) if mode == "f32r"
                               else P)
    wout_lay = W_out.reshape(KH, 128, V).transpose(1, 0, 2).reshape(128, KH * V)
    if mode == "bf16x3":
        wh_, wl_ = _split_bf16(wout_lay)
        shared["Wout_h"], shared["Wout_l"] = wh_, wl_
    else:
        shared["Wout"] = rnd(wout_lay)
    shared["benc"] = np.ascontiguousarray(b_enc.reshape(KH, 128).T)
    shared["bout"] = b_out.reshape(V, 1)
    shared["iota"] = np.tile(np.arange(V, dtype=f32), (128, NCH))
    shared["eye128"] = np.eye(128, dtype=f32)
    shared["eye6"] = np.eye(V, dtype=f32)

    in_maps = []
    for c in range(NC):
        m = dict(shared)
        xT = np.ascontiguousarray(x[c * BL:(c + 1) * BL].T)
        if mode == "bf16x3":
            m["xTh"], m["xTl"] = _split_bf16(xT)
        else:
            m["xT"] = rnd(xT)
        gc = gum[:, c * BL:(c + 1) * BL, :]
        m["g"] = np.ascontiguousarray(
            gc.reshape(T, NCH, 128, V).transpose(2, 0, 1, 3).reshape(128, T * 48))
        in_maps.append(m)
    return in_maps


def _run(inputs, mode=MM_MODE, trace=False):
    nc = build(mode)
    in_maps = _prep_inputs(inputs, mode)
    res = bass_utils.run_bass_kernel_spmd(nc, in_maps, list(range(NC)), trace=trace)
    msg = np.concatenate([r["msg"] for r in res.results], axis=0)
    logits = np.concatenate([r["logits"] for r in res.results], axis=1)
    ntok = np.concatenate([r["ntok"] for r in res.results], axis=0)
    return (msg, logits, ntok), res.exec_time_ns


def kernel(**inputs):
    last_err = None
    for attempt in range(3):
        try:
            out, _ = _run(inputs, MM_MODE, trace=False)
            return out
        except Exception as e:  # transient NRT device errors: retry
            last_err = e
    raise last_err


# revision 52
# speedup vs baseline: 1.0411x; 1.0062x over previous
"""Trainium2 Bass kernel for nn_AutoregressiveSender (GRU decoder + Gumbel-ST).

Self-contained: host-side prep (numpy) + SPMD Bass/Tile kernel on 8 NeuronCores.
Data parallel over batch (1024 rows/core), weights replicated.

Math notes vs reference:
- prev = token @ W_tok + b_tok is folded into the gate matmuls via
  P_g = W_tok @ Wg[:H] (6xH) and per-step bias vectors (host-precomputed),
  cutting gate contractions from 2H to H (+ a K=6 matmul).
- token == alpha * onehot(argmax(logits+g)) exactly, with
  alpha = (1-s)+s, s = 1/sum(exp(a-max)). Computed on-device.
- MM_MODE selects matmul precision:
    "f32"    exact fp32 (4 cyc/row, slowest)
    "bf16x3" hi/lo bf16 split, 3 passes (~5e-6 rel err)
    "f32r"   single-pass fp32r (~1.5e-4 rel err, fastest)
"""
import sys

for _p in ("/opt/trn_rl_repo", "/opt/pypackages"):
    if _p not in sys.path:
        sys.path.append(_p)

import numpy as np
import ml_dtypes
from contextlib import ExitStack

import concourse.tile as tile
from concourse import bacc, mybir
from concourse import bass_utils
from concourse.bass import _add_dep_helper

F32 = mybir.dt.float32
F32R = mybir.dt.float32r
BF16 = mybir.dt.bfloat16
AF = mybir.ActivationFunctionType
ALU = mybir.AluOpType

B, D, H, V, T = 8192, 2048, 1024, 6, 6
NC = 8
BL = B // NC          # 1024 rows per core
KD = D // 128         # 16 k-chunks (encoder)
KH = H // 128         # 8 k-chunks / m-chunks
BH = 2                # batch halves of 512
NB = BL // 2          # 512
NCH = BL // 128       # 8 batch chunks of 128

MM_MODE = "f32r"

_BUILD_CACHE = {}


def _split_bf16(a):
    hi = a.astype(ml_dtypes.bfloat16)
    lo = (a - hi.astype(np.float32)).astype(ml_dtypes.bfloat16)
    return hi, lo


def _round_f32r(a):
    """fp32r storage rounding: RNE to 11 mantissa bits (HW-verified)."""
    b = a.astype(np.float32).view(np.uint32).astype(np.uint64)
    shift = 12
    lsb = (b >> shift) & 1
    r = (b + ((1 << (shift - 1)) - 1 + lsb)) & ~np.uint64((1 << shift) - 1)
    return r.astype(np.uint32).view(np.float32)


def build(mode=MM_MODE):
    if mode in _BUILD_CACHE:
        return _BUILD_CACHE[mode]
    nc = bacc.Bacc("TRN2", target_bir_lowering=False, debug=False, num_devices=NC)
    dt_w = {"f32": F32, "bf16x3": BF16, "f32r": F32R}[mode]

    def din(name, shape, dt=F32):
        return nc.dram_tensor(name, shape, dt, kind="ExternalInput").ap()

    # per-core inputs
    if mode == "bf16x3":
        xTh_d = din("xTh", [D, BL], BF16)
        xTl_d = din("xTl", [D, BL], BF16)
    else:
        xT_d = din("xT", [D, BL], F32R if mode == "f32r" else F32)
    g_d = din("g", [128, T * 48])
    # replicated weights
    if mode == "bf16x3":
        wenc_h_d = din("Wenc_h", [D, H], BF16)
        wenc_l_d = din("Wenc_l", [D, H], BF16)
        gw_d = {g: (din(f"W{g}_h", [H, H], BF16), din(f"W{g}_l", [H, H], BF16))
                for g in "zrh"}
        p_d = {g: (din(f"P{g}_h", [V, H], BF16), din(f"P{g}_l", [V, H], BF16))
               for g in "zrh"}
        wout_h_d = din("Wout_h", [128, KH * V], BF16)
        wout_l_d = din("Wout_l", [128, KH * V], BF16)
    else:
        dt_in = F32R if mode == "f32r" else F32
        wenc_d = din("Wenc", [D, H], dt_in)
        gw_d = {g: din(f"W{g}", [H, H], dt_in) for g in "zrh"}
        p_d = {g: din(f"P{g}", [V, H], BF16 if mode == "f32r" else F32)
               for g in "zrh"}
        wout_d = din("Wout", [128, KH * V], dt_in)
    bias_d = din("bias", [128, 48])
    benc_d = din("benc", [128, KD // 2])
    bout_d = din("bout", [V, 1])
    iota_d = din("iota", [128, 48])
    eye128_d = din("eye128", [128, 128])
    eye6_d = din("eye6", [V, V])

    msg_d = nc.dram_tensor("msg", [BL, T * V], F32, kind="ExternalOutput").ap()
    log_d = nc.dram_tensor("logits", [T, BL, V], F32, kind="ExternalOutput").ap()
    ntok_d = nc.dram_tensor("ntok", [BL], F32, kind="ExternalOutput").ap()

    cast_dma = nc.sync  # f32r inputs are pre-rounded on host; no casting DMA
    dt_p = BF16 if mode == "f32r" else dt_w  # prev-token path dtype

    with tile.TileContext(nc) as tc, ExitStack() as ctx:
        # ---------- persistent SBUF ----------
        cons = ctx.enter_context(tc.tile_pool(name="cons", bufs=1))

        g_sb = cons.tile([128, T * 48], F32, tag="g")
        nc.sync.dma_start(g_sb[:], g_d)
        bias_sb = cons.tile([128, 48], F32, tag="bias")
        nc.sync.dma_start(bias_sb[:], bias_d)
        benc_sb = cons.tile([128, KD // 2], F32, tag="benc")
        nc.sync.dma_start(benc_sb[:], benc_d)
        bout_sb = cons.tile([V, 1], F32, tag="bout")
        nc.sync.dma_start(bout_sb[:], bout_d)
        iota_sb = cons.tile([128, 48], F32, tag="iota")
        nc.sync.dma_start(iota_sb[:], iota_d)
        eye128_sb = cons.tile([128, 128], F32, tag="eye128")
        nc.sync.dma_start(eye128_sb[:], eye128_d)
        eye6_sb = cons.tile([V, V], F32, tag="eye6")
        nc.sync.dma_start(eye6_sb[:], eye6_d)

        # state tiles
        st = ctx.enter_context(tc.tile_pool(name="state", bufs=1))
        h_sb = st.tile([128, KH, BL], F32, tag="h")
        rc = st.tile([128, NCH], F32, tag="rc")
        nt = st.tile([128, NCH], F32, tag="nt")
        msg_sb = st.tile([128, NCH, T * V], F32, tag="msg")
        tokT = st.tile([V, BL], dt_p, tag="tokT")
        nc.vector.memset(rc[:], 1.0)
        nc.vector.memset(nt[:], 0.0)

        # ---------- encoder ----------
        # W_enc is loaded into SBUF once (row-contiguous DMAs) and reused for
        # both batch halves; xT chunks stream per half.
        KHALF = KD // 2
        with tc.tile_pool(name="encps", bufs=KH, space="PSUM") as encps, \
             tc.tile_pool(name="encx", bufs=6) as encx, \
             tc.tile_pool(name="encw", bufs=2) as encw:
            if mode == "bf16x3":
                wt_h = [encw.tile([128, KHALF, H], BF16, tag="weh", name=f"weh{h}")
                        for h in range(2)]
                wt_l = [encw.tile([128, KHALF, H], BF16, tag="wel", name=f"wel{h}")
                        for h in range(2)]

                def load_wrow(half, kk):
                    k = half * KHALF + kk
                    nc.sync.dma_start(wt_h[half][:, kk, :],
                                      wenc_h_d[k * 128:(k + 1) * 128, :])
                    nc.sync.dma_start(wt_l[half][:, kk, :],
                                      wenc_l_d[k * 128:(k + 1) * 128, :])
            else:
                wts_enc = [encw.tile([128, KHALF, H], dt_w, tag="we", name=f"we{h}")
                           for h in range(2)]

                def load_wrow(half, kk):
                    k = half * KHALF + kk
                    for hh2 in range(2):
                        cast_dma.dma_start(
                            wts_enc[half][:, kk, hh2 * (H // 2):(hh2 + 1) * (H // 2)],
                            wenc_d[k * 128:(k + 1) * 128,
                                   hh2 * (H // 2):(hh2 + 1) * (H // 2)])

            for kk in range(KHALF):
                load_wrow(0, kk)
            for half in range(2):
                k0 = half * KHALF
                for bh in range(BH):
                    pts = [encps.tile([128, NB], F32, tag="enc",
                                      name=f"encp{half}_{bh}_{i}") for i in range(KH)]
                    for kk in range(KHALF):
                        k = k0 + kk
                        if mode == "bf16x3":
                            xh = encx.tile([128, NB], BF16, tag="xh")
                            nc.sync.dma_start(
                                xh[:], xTh_d[k * 128:(k + 1) * 128, bh * NB:(bh + 1) * NB])
                            xl = encx.tile([128, NB], BF16, tag="xl")
                            last_enc_dma = nc.sync.dma_start(
                                xl[:], xTl_d[k * 128:(k + 1) * 128, bh * NB:(bh + 1) * NB])
                        else:
                            xk = encx.tile([128, NB], dt_w, tag="x")
                            last_enc_dma = cast_dma.dma_start(
                                xk[:], xT_d[k * 128:(k + 1) * 128, bh * NB:(bh + 1) * NB])
                        if half == 0 and bh == 1:
                            load_wrow(1, kk)   # prefetch half1 weights in-stream
                        for m in range(KH):
                            first = (kk == 0)
                            last = (kk == KHALF - 1)
                            ms = slice(m * 128, (m + 1) * 128)
                            if mode == "bf16x3":
                                nc.tensor.matmul(pts[m][:], wt_h[half][:, kk, ms], xh[:], start=first, stop=False)
                                nc.tensor.matmul(pts[m][:], wt_h[half][:, kk, ms], xl[:], start=False, stop=False)
                                nc.tensor.matmul(pts[m][:], wt_l[half][:, kk, ms], xh[:], start=False, stop=last)
                            else:
                                nc.tensor.matmul(pts[m][:], wts_enc[half][:, kk, ms], xk[:], start=first, stop=last)
                    for m in range(KH):
                        hdst = h_sb[:, m, bh * NB:(bh + 1) * NB]
                        if half == 0:
                            # split evacs across ACT/DVE to halve the
                            # PSUM-free latency between encoder sub-phases
                            if m % 2 == 0:
                                nc.scalar.activation(hdst, pts[m][:], AF.Identity,
                                                     bias=benc_sb[:, m:m + 1])
                            else:
                                nc.vector.scalar_tensor_tensor(
                                    hdst, pts[m][:], 1.0, benc_sb[:, m:m + 1]
                                    .broadcast_to((128, NB)),
                                    op0=ALU.mult, op1=ALU.add)
                        else:
                            nc.vector.tensor_tensor(hdst, hdst, pts[m][:], op=ALU.add)

        wpool = ctx.enter_context(tc.tile_pool(name="wts", bufs=1))
        # GRU weights resident: [p, k, m] with k = contraction chunk.
        # Loaded in phase order (r, h, z) and gated behind the encoder's DMAs
        # so they don't steal HBM bandwidth from the encoder's working set.
        def after_enc(inst):
            _add_dep_helper(inst.ins, last_enc_dma.ins, sync=True,
                            reason="gate weights after encoder DMA")
            return inst

        gw = {}
        for g in "rhz":
            if mode == "bf16x3":
                wh = wpool.tile([128, KH, H], BF16, tag=f"W{g}h")
                wl = wpool.tile([128, KH, H], BF16, tag=f"W{g}l")
                after_enc(nc.sync.dma_start(
                    wh[:], gw_d[g][0].rearrange("(k p) m -> p k m", p=128)))
                after_enc(nc.sync.dma_start(
                    wl[:], gw_d[g][1].rearrange("(k p) m -> p k m", p=128)))
                gw[g] = (wh, wl)
            else:
                w = wpool.tile([128, KH, H], dt_w, tag=f"W{g}")
                after_enc(cast_dma.dma_start(
                    w[:], gw_d[g].rearrange("(k p) m -> p k m", p=128)))
                gw[g] = w
        pw = {}
        for g in "zrh":
            if mode == "bf16x3":
                ph = cons.tile([V, H], BF16, tag=f"P{g}h")
                pl = cons.tile([V, H], BF16, tag=f"P{g}l")
                nc.sync.dma_start(ph[:], p_d[g][0])
                nc.sync.dma_start(pl[:], p_d[g][1])
                pw[g] = (ph, pl)
            else:
                p = cons.tile([V, H], dt_p, tag=f"P{g}")
                nc.sync.dma_start(p[:], p_d[g])
                pw[g] = p
        if mode == "bf16x3":
            wout_h = cons.tile([128, KH * V], BF16, tag="wouth")
            wout_l = cons.tile([128, KH * V], BF16, tag="woutl")
            nc.sync.dma_start(wout_h[:], wout_h_d)
            nc.sync.dma_start(wout_l[:], wout_l_d)
        else:
            wout = cons.tile([128, KH * V], dt_w, tag="wout")
            cast_dma.dma_start(wout[:], wout_d)

        # ---------- decode steps ----------
        persist = (mode == "f32r")
        ps = ctx.enter_context(tc.tile_pool(name="ps", bufs=5, space="PSUM"))
        lps = ctx.enter_context(tc.tile_pool(name="lps", bufs=1, space="PSUM"))
        tps = ctx.enter_context(tc.tile_pool(name="tps", bufs=2, space="PSUM"))
        hs = ctx.enter_context(tc.tile_pool(
            name="hsplit", bufs=(17 if persist else 9)))
        rhp = ctx.enter_context(tc.tile_pool(name="rhp", bufs=8))
        gp = ctx.enter_context(tc.tile_pool(name="gates", bufs=2))
        rhf = None if mode == "f32r" else ctx.enter_context(tc.tile_pool(
            name="rhf", bufs=(10 if mode == "f32" else 2)))
        htp = ctx.enter_context(tc.tile_pool(name="htp", bufs=2))
        sp = ctx.enter_context(tc.tile_pool(name="smax", bufs=1))

        def split_one(bh, k):
            """snapshot h[:, k, bh] as matmul rhs (pre-rounded for the MM dtype)."""
            src = h_sb[:, k, bh * NB:(bh + 1) * NB]
            if mode == "bf16x3":
                hh = hs.tile([128, NB], BF16, tag="hh")
                nc.vector.tensor_copy(hh[:], src)
                hl = hs.tile([128, NB], BF16, tag="hl")
                nc.vector.tensor_tensor(hl[:], src, hh[:], op=ALU.subtract)
                return (hh[:], hl[:])
            elif mode == "f32r":
                hr = hs.tile([128, NB], F32R, tag="hr")
                nc.vector.tensor_copy(hr[:], src)
                return hr[:]
            else:
                # snapshot: z-gate MMs must see pre-update h
                hc = hs.tile([128, NB], F32, tag="hc")
                nc.vector.tensor_copy(hc[:], src)
                return hc[:]

        def mk_hsplit(bh):
            return [split_one(bh, k) for k in range(KH)]

        def gate_mms(pt, g, m, rhs_split, t, emit_p_inline=True):
            """accumulate gate matmuls for output chunk m into psum pt.

            The K=6 prev-token matmul goes LAST so the h-part matmuls can
            start before tokenT (previous step softmax) is ready.
            """
            has_p = t > 0
            for k in range(KH):
                first = (k == 0)
                last = (k == KH - 1) and not has_p
                if mode == "bf16x3":
                    wh, wl = gw[g]
                    hh, hl = rhs_split[k]
                    lw_h = wh[:, k, m * 128:(m + 1) * 128]
                    lw_l = wl[:, k, m * 128:(m + 1) * 128]
                    nc.tensor.matmul(pt[:], lw_h, hh, start=first, stop=False)
                    nc.tensor.matmul(pt[:], lw_h, hl, start=False, stop=False)
                    nc.tensor.matmul(pt[:], lw_l, hh, start=False, stop=last)
                else:
                    w = gw[g]
                    nc.tensor.matmul(pt[:], w[:, k, m * 128:(m + 1) * 128],
                                     rhs_split[k], start=first, stop=last)
            if has_p and emit_p_inline:
                emit_p(pt, g, m)

        def emit_p(pt, g, m):
            ts = tokT[:, bh_cur * NB:(bh_cur + 1) * NB]
            if mode == "bf16x3":
                ph, pl = pw[g]
                nc.tensor.matmul(pt[:], ph[:, m * 128:(m + 1) * 128], ts,
                                 start=False, stop=False)
                nc.tensor.matmul(pt[:], pl[:, m * 128:(m + 1) * 128], ts,
                                 start=False, stop=True)
            else:
                nc.tensor.matmul(pt[:], pw[g][:, m * 128:(m + 1) * 128], ts,
                                 start=False, stop=True)

        cur_split = {bh: mk_hsplit(bh) for bh in range(BH)} if persist else {}
        for t in range(T):
            logit_sb = []
            for bh in range(BH):
                bh_cur = bh
                hsplit = cur_split[bh] if persist else mk_hsplit(bh)
                new_split = [None] * KH
                # r gates + rh products (+ splits). The K=6 prev-token matmuls
                # of the first DEFER_N groups are deferred to the phase end so
                # the PE never stalls on tokenT (previous step's softmax).
                rh_split = [None] * KH

                def r_evac_rh(pt, m):
                    bcol_r = (1 if t == 0 else 4) * KH + m
                    r_m = gp.tile([128, NB], F32, tag="r", name=f"r_{t}_{bh}_{m}")
                    nc.scalar.activation(r_m[:], pt[:], AF.Sigmoid,
                                         bias=bias_sb[:, bcol_r:bcol_r + 1])
                    if mode == "f32r":
                        rhr = rhp.tile([128, NB], F32R, tag="rhr",
                                       name=f"rhr_{t}_{bh}_{m}")
                        nc.vector.tensor_tensor(rhr[:], r_m[:],
                                                h_sb[:, m, bh * NB:(bh + 1) * NB], op=ALU.mult)
                        rh_split[m] = rhr[:]
                    else:
                        rh_m = rhf.tile([128, NB], F32, tag="rh",
                                        name=f"rh_{t}_{bh}_{m}")
                        nc.vector.tensor_tensor(rh_m[:], r_m[:],
                                                h_sb[:, m, bh * NB:(bh + 1) * NB], op=ALU.mult)
                        if mode == "bf16x3":
                            rhh = rhp.tile([128, NB], BF16, tag="rhh",
                                           name=f"rhh_{t}_{bh}_{m}")
                            nc.vector.tensor_copy(rhh[:], rh_m[:])
                            rhl = rhp.tile([128, NB], BF16, tag="rhl",
                                           name=f"rhl_{t}_{bh}_{m}")
                            nc.vector.tensor_tensor(rhl[:], rh_m[:], rhh[:], op=ALU.subtract)
                            rh_split[m] = (rhh[:], rhl[:])
                        else:
                            rh_split[m] = rh_m[:]

                # process r-groups in pairs so their two bf16 prev-token
                # matmuls are adjacent (second weight load pipelines)
                for m0 in range(0, KH, 2):
                    pair = []
                    for m in (m0, m0 + 1):
                        pt = ps.tile([128, NB], F32, tag="mm")
                        gate_mms(pt, "r", m, hsplit, t, emit_p_inline=False)
                        pair.append((pt, m))
                    if t > 0:
                        for pt, m in pair:
                            emit_p(pt, "r", m)
                    for pt, m in pair:
                        r_evac_rh(pt, m)
                # h_tilde + z + h update, per m. The two bf16 K=6
                # prev-token matmuls are emitted adjacently so the second's
                # weight load pipelines (no fp32r->bf16 switch between them).
                for m in range(KH):
                    pt_h = ps.tile([128, NB], F32, tag="mm")
                    gate_mms(pt_h, "h", m, rh_split, t, emit_p_inline=False)
                    pt = ps.tile([128, NB], F32, tag="mm")
                    gate_mms(pt, "z", m, hsplit, t, emit_p_inline=False)
                    if t > 0:
                        emit_p(pt_h, "h", m)
                        emit_p(pt, "z", m)
                    bcol_h = (2 if t == 0 else 5) * KH + m
                    ht_m = htp.tile([128, NB], F32, tag="ht")
                    nc.scalar.activation(ht_m[:], pt_h[:], AF.Tanh,
                                         bias=bias_sb[:, bcol_h:bcol_h + 1])
                    bcol_z = (0 if t == 0 else 3) * KH + m
                    z_m = gp.tile([128, NB], F32, tag="z")
                    nc.scalar.activation(z_m[:], pt[:], AF.Sigmoid,
                                         bias=bias_sb[:, bcol_z:bcol_z + 1])
                    hcur = h_sb[:, m, bh * NB:(bh + 1) * NB]
                    nc.vector.tensor_tensor(ht_m[:], ht_m[:], hcur, op=ALU.subtract)
                    nc.vector.tensor_tensor(ht_m[:], ht_m[:], z_m[:], op=ALU.mult)
                    nc.vector.tensor_tensor(hcur, hcur, ht_m[:], op=ALU.add)
                    if persist:
                        new_split[m] = split_one(bh, m)
                # logits for this bh (uses updated h)
                if persist:
                    cur_split[bh] = new_split
                    hsplit2 = new_split
                else:
                    hsplit2 = mk_hsplit(bh)
                pl_t = lps.tile([V, NB], F32, tag="lg")
                for k in range(KH):
                    first = (k == 0)
                    last = (k == KH - 1)
                    if mode == "bf16x3":
                        hh, hl = hsplit2[k]
                        nc.tensor.matmul(pl_t[:], wout_h[:, k * V:(k + 1) * V], hh,
                                         start=first, stop=False)
                        nc.tensor.matmul(pl_t[:], wout_h[:, k * V:(k + 1) * V], hl,
                                         start=False, stop=False)
                        nc.tensor.matmul(pl_t[:], wout_l[:, k * V:(k + 1) * V], hh,
                                         start=False, stop=last)
                    else:
                        nc.tensor.matmul(pl_t[:], wout[:, k * V:(k + 1) * V],
                                         hsplit2[k], start=first, stop=last)
                lsb = sp.tile([V, NB], F32, tag="lsb", bufs=2)
                nc.scalar.activation(lsb[:], pl_t[:], AF.Identity, bias=bout_sb[:])
                logit_sb.append(lsb)

            # ---- transpose logits to batch-major, (128, NCH, V)
            lbm = sp.tile([128, NCH, V], F32, tag="lbm")
            for c in range(NCH):
                bh, cc = divmod(c, NCH // 2)
                ptt = tps.tile([128, V], F32, tag="tp")
                nc.tensor.transpose(ptt[:], logit_sb[bh][:, cc * 128:(cc + 1) * 128],
                                    eye6_sb[:])
                nc.scalar.activation(lbm[:, c, :], ptt[:], AF.Copy)
            nc.sync.dma_start(log_d[t].rearrange("(c p) v -> p c v", p=128), lbm[:])

            # ---- a = logits + g_t ; softmax/argmax/alpha/token
            a_t = sp.tile([128, 48], F32, tag="a")
            a3 = a_t[:].rearrange("p (c v) -> p c v", v=V)
            g3 = g_sb[:, t * 48:(t + 1) * 48].rearrange("p (c v) -> p c v", v=V)
            nc.vector.tensor_tensor(a3, lbm[:], g3, op=ALU.add)
            m8 = sp.tile([128, NCH], F32, tag="m8")
            nc.vector.tensor_reduce(m8[:], a3, axis=mybir.AxisListType.X, op=ALU.max)
            m8b = m8[:].rearrange("p (c o) -> p c o", o=1).broadcast_to((128, NCH, V))
            eqv = sp.tile([128, 48], F32, tag="eqv")
            eq3 = eqv[:].rearrange("p (c v) -> p c v", v=V)
            nc.vector.tensor_tensor(eq3, a3, m8b, op=ALU.is_ge)
            # s = 1/sum(exp(a-m)); alpha = (1-s)+s
            sub = sp.tile([128, 48], F32, tag="sub")
            nc.vector.tensor_tensor(
                sub[:].rearrange("p (c v) -> p c v", v=V), a3, m8b, op=ALU.subtract)
            ex = sp.tile([128, 48], F32, tag="ex")
            nc.scalar.activation(ex[:], sub[:], AF.Exp)
            S8 = sp.tile([128, NCH], F32, tag="S8")
            nc.vector.tensor_reduce(S8[:], ex[:].rearrange("p (c v) -> p c v", v=V),
                                    axis=mybir.AxisListType.X, op=ALU.add)
            s8 = sp.tile([128, NCH], F32, tag="s8")
            nc.vector.reciprocal(s8[:], S8[:])
            al = sp.tile([128, NCH], F32, tag="al")
            nc.vector.tensor_scalar(al[:], s8[:], -1.0, 1.0, op0=ALU.mult, op1=ALU.add)
            nc.vector.tensor_tensor(al[:], al[:], s8[:], op=ALU.add)
            # first-argmax one-hot via iota/min
            mi = sp.tile([128, 48], F32, tag="mi")
            nc.vector.memset(mi[:], 64.0)
            nc.vector.copy_predicated(mi[:], eqv[:].bitcast(mybir.dt.uint32), iota_sb[:])
            idx8 = sp.tile([128, NCH], F32, tag="idx8")
            nc.vector.tensor_reduce(idx8[:], mi[:].rearrange("p (c v) -> p c v", v=V),
                                    axis=mybir.AxisListType.X, op=ALU.min)
            idxb = idx8[:].rearrange("p (c o) -> p c o", o=1).broadcast_to((128, NCH, V))
            tok = sp.tile([128, 48], F32, tag="tok")
            tok3 = tok[:].rearrange("p (c v) -> p c v", v=V)
            nc.vector.tensor_tensor(tok3, iota_sb[:].rearrange("p (c v) -> p c v", v=V),
                                    idxb, op=ALU.is_equal)
            alb = al[:].rearrange("p (c o) -> p c o", o=1).broadcast_to((128, NCH, V))
            nc.vector.tensor_tensor(tok3, tok3, alb, op=ALU.mult)
            # nt += rc ; masked = tok * rc ; rc *= (1 - tok[:,:,V-1])
            nc.vector.tensor_tensor(nt[:], nt[:], rc[:], op=ALU.add)
            rcb = rc[:].rearrange("p (c o) -> p c o", o=1).broadcast_to((128, NCH, V))
            nc.vector.tensor_tensor(
                msg_sb[:].rearrange("p c (t v) -> p c t v", v=V)[:, :, t, :],
                tok3, rcb, op=ALU.mult)
            tl8 = sp.tile([128, NCH], F32, tag="tl8")
            nc.vector.tensor_scalar(tl8[:], tok3[:, :, V - 1], -1.0, 1.0,
                                    op0=ALU.mult, op1=ALU.add)
            nc.vector.tensor_tensor(rc[:], rc[:], tl8[:], op=ALU.mult)
            # tokenT for next step
            if t < T - 1:
                for c in range(NCH):
                    ptt = tps.tile([V, 128], F32, tag="tp")
                    nc.tensor.transpose(ptt[:], tok[:, c * V:(c + 1) * V], eye128_sb[:])
                    nc.scalar.activation(tokT[:, c * 128:(c + 1) * 128], ptt[:], AF.Copy)

        # ---------- outputs ----------
        nc.sync.dma_start(msg_d.rearrange("(c p) w -> p c w", p=128), msg_sb[:])
        nc.sync.dma_start(ntok_d.rearrange("(c p) -> p c", p=128), nt[:])

    nc.compile()
    _BUILD_CACHE[mode] = nc
    return nc


def _prep_inputs(inputs, mode):
    f32 = np.float32
    x = np.asarray(inputs["x"], f32)
    u = np.asarray(inputs["u_noise"], f32)
    W_enc = np.asarray(inputs["W_enc"], f32)
    b_enc = np.asarray(inputs["b_enc"], f32)
    start_embed = np.asarray(inputs["start_embed"], f32)
    W_tok = np.asarray(inputs["W_tok"], f32)
    b_tok = np.asarray(inputs["b_tok"], f32)
    Wg = {g: np.asarray(inputs["W" + g], f32) for g in "zrh"}
    bg = {g: np.asarray(inputs["b" + g], f32) for g in "zrh"}
    W_out = np.asarray(inputs["W_out"], f32)
    b_out = np.asarray(inputs["b_out"], f32)

    eps = f32(1e-10)
    gum = -np.log(-np.log(u + eps) + eps)  # fp32 throughout

    rnd = _round_f32r if mode == "f32r" else (lambda a: a)
    shared = {}
    if mode == "bf16x3":
        eh, el = _split_bf16(W_enc)
        shared["Wenc_h"], shared["Wenc_l"] = eh, el
    else:
        shared["Wenc"] = rnd(W_enc)
    bias_vecs = []
    for vset in (0, 1):  # 0: step0 (start_embed), 1: steps>=1 (b_tok)
        src = start_embed if vset == 0 else b_tok
        for g in "zrh":
            top = Wg[g][:H]
            vec = (src.astype(np.float64) @ top.astype(np.float64)
                   + bg[g].astype(np.float64)).astype(f32)
            bias_vecs.append(vec)
    bias_lay = np.zeros((128, 48), f32)
    for i, vec in enumerate(bias_vecs):
        bias_lay[:, i * KH:(i + 1) * KH] = vec.reshape(KH, 128).T
    shared["bias"] = bias_lay
    for g in "zrh":
        bot = Wg[g][H:]
        P = (W_tok.astype(np.float64) @ Wg[g][:H].astype(np.float64)).astype(f32)
        if mode == "bf16x3":
            bh_, bl_ = _split_bf16(bot)
            shared[f"W{g}_h"], shared[f"W{g}_l"] = bh_, bl_
            ph_, pl_ = _split_bf16(P)
            shared[f"P{g}_h"], shared[f"P{g}_l"] = ph_, pl_
        else:
            shared[f"W{g}"] = rnd(bot)
            shared[f"P{g}"] = (P.astype(ml_dtypes.bfloat16) if mode == "f32r"
                               else P)
    wout_lay = W_out.reshape(KH, 128, V).transpose(1, 0, 2).reshape(128, KH * V)
    if mode == "bf16x3":
        wh_, wl_ = _split_bf16(wout_lay)
        shared["Wout_h"], shared["Wout_l"] = wh_, wl_
    else:
        shared["Wout"] = rnd(wout_lay)
    shared["benc"] = np.ascontiguousarray(b_enc.reshape(KH, 128).T)
    shared["bout"] = b_out.reshape(V, 1)
    shared["iota"] = np.tile(np.arange(V, dtype=f32), (128, NCH))
    shared["eye128"] = np.eye(128, dtype=f32)
    shared["eye6"] = np.eye(V, dtype=f32)

    in_maps = []
    for c in range(NC):
        m = dict(shared)
        xT = np.ascontiguousarray(x[c * BL:(c + 1) * BL].T)
        if mode == "bf16x3":
            m["xTh"], m["xTl"] = _split_bf16(xT)
        else:
            m["xT"] = rnd(xT)
        gc = gum[:, c * BL:(c + 1) * BL, :]
        m["g"] = np.ascontiguousarray(
            gc.reshape(T, NCH, 128, V).transpose(2, 0, 1, 3).reshape(128, T * 48))
        in_maps.append(m)
    return in_maps


def _run(inputs, mode=MM_MODE, trace=False):
    nc = build(mode)
    in_maps = _prep_inputs(inputs, mode)
    res = bass_utils.run_bass_kernel_spmd(nc, in_maps, list(range(NC)), trace=trace)
    msg = np.concatenate([r["msg"] for r in res.results], axis=0)
    logits = np.concatenate([r["logits"] for r in res.results], axis=1)
    ntok = np.concatenate([r["ntok"] for r in res.results], axis=0)
    return (msg, logits, ntok), res.exec_time_ns


def kernel(**inputs):
    last_err = None
    for attempt in range(3):
        try:
            out, _ = _run(inputs, MM_MODE, trace=False)
            return out
        except Exception as e:  # transient NRT device errors: retry
            last_err = e
    raise last_err
